# revision 53
# baseline (speedup 1.0000x reference)
"""Trainium2 Bass kernel for nn_GCNN_mutual_attention — v3.

Strategy (8 NeuronCores, SPMD single program, per-core input slices):
  - GCN branches sharded 2 graphs x 2 feature-halves x 2 dst-window-halves:
    core (g, fh, wh) computes table g=[16000,512] fp8 via fp8 DoubleRow
    matmuls (x and W pre-quantized to fp8e4, W scaled by 256 with the
    inverse folded into the per-row dinv scale), then aggregates the edges
    whose dst lies in its window half. dma_gather uses 512B fp8 rows on 4
    SWDGE queues (descriptor-rate bound). Scatter via fp8 one-hot matmuls,
    consecutive same-window tile pairs fused with DoubleRow (2 k-tiles per
    PE instruction). One-hot tiles are built one gather-chunk ahead so the
    scatter's DVE work never waits behind transformer DVE ops.
  - Transformer branch batch-sharded (4 slots/core, rank-strided). Heads
    packed in pairs at partition offsets {0,32} (PE base-partition limit);
    per-head softmax numerator+denominator accumulate in one [33,T] PSUM
    tile via a ones column at partition 32; head outputs collect in SBUF
    (on_all, 32-partition head stride) and a single matmul applies
    attn_out per (slot, layer); the 4 denominators broadcast with one
    selection matmul. Emission is generator-based: two slot-chains run in
    lockstep micro-steps (matching the bufs=2 tile rings) and alternate
    with the other pair at layer boundaries, interleaved into the gather
    chunk stream so PE/ACT/DVE fill the gather window without serial tails.
  - Tiny second launch reduces the 4-core fc partials per graph + head.
"""
import numpy as np
import ml_dtypes
from contextlib import ExitStack

import jax
from jax.sharding import Mesh, PartitionSpec
from jax.experimental.shard_map import shard_map

import concourse.bass as bass
import concourse.tile as tile
import concourse.mybir as mybir
from concourse import bacc
from concourse.bass2jax import _bass_exec_p, install_neuronx_cc_hook, partition_id_tensor
from concourse.masks import make_identity

BF16 = mybir.dt.bfloat16
FP8 = mybir.dt.float8e4
F32 = mybir.dt.float32
I16 = mybir.dt.int16
Alu = mybir.AluOpType
Act = mybir.ActivationFunctionType
X = mybir.AxisListType.X
DR = mybir.MatmulPerfMode.DoubleRow
bf16 = ml_dtypes.bfloat16
fp8 = ml_dtypes.float8_e4m3
WSCALE = 256.0           # fp8 weight pre-scale (keeps 0.02-scale weights normal)

# problem constants
N, F, E, B, OD = 16000, 1024, 256000, 32, 128
DD, TD, NH, DH, DFF, NL = 80, 32, 4, 8, 128, 2
LSUB, MAXLEN = 128, 512
NEG, SLOPE, EPS = -1e9, 0.01, 1e-5
NC = 8
FC = 512                 # feature chunk per core (feat half)
CPG = 4                  # cores per graph
NW = N // 128            # 125 node windows
GPC = B // NC            # transformer graphs per core
TPC = 16                 # gather tiles per chunk (16*128 idx = 2048)
OHB = 8                  # one-hot tiles batched per DVE instr
GDT = FP8                # gather table dtype (BF16 or FP8)
ISQ = float(1.0 / np.sqrt(DH))

_runner_cache = {}


# --------------------------------------------------------------------------
# SPMD runner (reused from baseline)
# --------------------------------------------------------------------------
class _SpmdRunner:
    def __init__(self, nc, n_cores=NC):
        install_neuronx_cc_hook()
        self.n_cores = n_cores
        in_names, out_names, out_avals, zero_outs = [], [], [], []
        pname = nc.partition_id_tensor.name if nc.partition_id_tensor else None
        for alloc in nc.m.functions[0].allocations:
            if not isinstance(alloc, mybir.MemoryLocationSet):
                continue
            name = alloc.memorylocations[0].name
            if alloc.kind == "ExternalInput":
                if name != pname:
                    in_names.append(name)
            elif alloc.kind == "ExternalOutput":
                out_names.append(name)
                out_avals.append(jax.core.ShapedArray(
                    tuple(alloc.tensor_shape), mybir.dt.np(alloc.dtype)))
                zero_outs.append(np.zeros(tuple(alloc.tensor_shape),
                                          mybir.dt.np(alloc.dtype)))
        self.in_names, self.out_names = in_names, out_names
        self.out_avals, self.zero_outs = out_avals, zero_outs
        n_params, n_outs = len(in_names), len(out_avals)
        all_in = list(in_names) + list(out_names)
        if pname is not None:
            all_in.append(pname)

        def _body(*args):
            operands = list(args)
            if pname is not None:
                operands.append(partition_id_tensor())
            return tuple(_bass_exec_p.bind(
                *operands, out_avals=tuple(out_avals), in_names=tuple(all_in),
                out_names=tuple(out_names), lowering_input_output_aliases=(),
                sim_require_finite=True, sim_require_nnan=True, nc=nc))

        devices = jax.devices()[:n_cores]
        self.mesh = Mesh(np.asarray(devices), ("core",))
        in_specs = (PartitionSpec("core"),) * (n_params + n_outs)
        out_specs = (PartitionSpec("core"),) * n_outs
        self.fn = jax.jit(
            shard_map(_body, mesh=self.mesh, in_specs=in_specs,
                      out_specs=out_specs, check_rep=False),
            keep_unused=True)
        self.n_params = n_params

    def prep(self, in_maps):
        per_core = [[np.asarray(m[n]) for n in self.in_names] for m in in_maps]
        concat_in = [np.concatenate([per_core[c][i] for c in range(self.n_cores)],
                                    axis=0) for i in range(self.n_params)]
        concat_zeros = [np.zeros((self.n_cores * z.shape[0], *z.shape[1:]), z.dtype)
                        for z in self.zero_outs]
        return concat_in, concat_zeros

    def run(self, in_maps):
        concat_in, concat_zeros = self.prep(in_maps)
        out_arrs = self.fn(*concat_in, *concat_zeros)
        return [
            {name: np.asarray(out_arrs[i]).reshape(self.n_cores,
                                                   *self.out_avals[i].shape)[c]
             for i, name in enumerate(self.out_names)}
            for c in range(self.n_cores)
        ]


# --------------------------------------------------------------------------
# host-side preprocessing
# --------------------------------------------------------------------------
def _edge_sort(ei):
    """dst-sorted edges incl. self loops, split stats per window."""
    src = np.asarray(ei[0], np.int64)
    dst = np.asarray(ei[1], np.int64)
    deg = np.bincount(dst, minlength=N).astype(np.float64) + 1.0
    dinv = (1.0 / np.sqrt(deg)).astype(np.float32)
    sl = np.arange(N, dtype=np.int64)
    src = np.concatenate([src, sl])
    dst = np.concatenate([dst, sl])
    order = np.argsort(dst, kind="stable")
    s_s, d_s = src[order], dst[order]
    counts = np.bincount(d_s >> 7, minlength=NW)
    return dict(s=s_s, d=d_s, counts=counts, dinv=dinv)


def _half_stream(g, wins, tpp, NT_H):
    """Pack the edges of window list `wins` into the common padded layout.

    tpp[p] = tiles for position p (0 for dummy). Returns idx, dstrel streams.
    """
    ne_pad = NT_H * 128
    src_stream = np.zeros(ne_pad, np.int16)
    dstrel_stream = np.full(ne_pad, -1.0, np.float32)
    off = np.concatenate([[0], np.cumsum(g["counts"])])
    pos = 0
    for p, w in enumerate(wins):
        if w >= 0:
            c = int(g["counts"][w])
            a, b = int(off[w]), int(off[w + 1])
            so = np.argsort(g["s"][a:b], kind="stable")
            src_stream[pos:pos + c] = g["s"][a:b][so]
            dstrel_stream[pos:pos + c] = (g["d"][a:b][so] - (w << 7)).astype(
                np.float32)
        pos += int(tpp[p]) * 128
    idx_np = np.tile(src_stream.reshape(-1, 16).T, (8, 1)).copy()
    dstrel_np = np.ascontiguousarray(
        dstrel_stream.reshape(NT_H, 128).T).astype(bf16)
    return idx_np, dstrel_np


def _host_prep(inp):
    inp = {k: np.asarray(v) for k, v in inp.items()}
    g1 = _edge_sort(inp["pro1_edge_index"])
    g2 = _edge_sort(inp["pro2_edge_index"])
    # split point balancing edge counts (common across graphs)
    cum = np.cumsum(g1["counts"] + g2["counts"])
    WS = int(np.argmin(np.abs(cum - cum[-1] / 2))) + 1
    winsA = list(range(WS))
    winsB = list(range(WS, NW))
    P = max(len(winsA), len(winsB))
    winsA += [-1] * (P - len(winsA))          # dummy positions at end
    winsB += [-1] * (P - len(winsB))
    # common tiles-per-position (dummies get 1 padding tile)
    tpp = np.zeros(P, np.int64)
    for p in range(P):
        cands = []
        for g, wins in ((g1, winsA), (g2, winsB), (g1, winsB), (g2, winsA)):
            w = wins[p]
            cands.append(1 if w < 0 else (int(g["counts"][w]) + 127) // 128)
        tpp[p] = max(cands)
    ntiles = int(tpp.sum())
    NT = ((ntiles + TPC - 1) // TPC) * TPC
    pos_of_tile = np.full(NT, -1, np.int64)
    t = 0
    for p in range(P):
        n = int(tpp[p])
        pos_of_tile[t:t + n] = p
        t += n
    streams = {}
    for gi, g in ((0, g1), (1, g2)):
        for hi, wins in ((0, winsA), (1, winsB)):
            streams[(gi, hi)] = _half_stream(g, wins, tpp, NT)

    def tile_xT(x):
        xT = np.ascontiguousarray(x.T.astype(fp8))             # [F, N]
        tt = xT.reshape(8, 128, NW, 128)                       # [kk, p, m, j]
        return np.ascontiguousarray(tt.transpose(2, 1, 0, 3))  # [m, p, kk, j]

    xtp = [tile_xT(inp["pro1_x"]), tile_xT(inp["pro2_x"])]
    batch = [np.asarray(inp["pro1_batch"], np.int64),
             np.asarray(inp["pro2_batch"], np.int64)]
    gcn_w = [inp["gcn1_w"], inp["gcn2_w"]]
    gcn_b = [inp["gcn1_b"], inp["gcn2_b"]]
    fc_w = [inp["fc1_w"], inp["fc2_w"]]
    dinv = [g1["dinv"], g2["dinv"]]
    wins_of = [winsA, winsB]

    # transformer slot assignment (rank-strided, common padded T per slot)
    lens = np.stack([np.asarray(inp[k + "_lengths"], np.int64) for k in
                     ("mas1_straight", "mas1_flipped", "mas2_straight",
                      "mas2_flipped")])
    L = lens.sum(0)
    rank = np.argsort(-L, kind="stable")
    slot_graphs = [[int(rank[s * NC + c]) for c in range(NC)] for s in range(GPC)]
    Ts = [int(min(MAXLEN, ((int(L[rank[s * NC]]) + 127) // 128) * 128))
          for s in range(GPC)]

    inds = ((1.0, 1.0), (0.0, 1.0), (1.0, 0.0), (0.0, 0.0))
    mas_names = ("mas1_straight", "mas1_flipped", "mas2_straight", "mas2_flipped")

    per_core = [dict() for _ in range(NC)]
    for c in range(NC):
        m = per_core[c]
        gi = c // CPG
        j = c % CPG
        fh, wh = j // 2, j % 2
        sl = slice(fh * FC, (fh + 1) * FC)
        wins = wins_of[wh]
        m["xtp"] = xtp[gi]
        m["idx"], m["dstrel"] = streams[(gi, wh)]
        dcols = np.zeros((128, P), np.float32)
        bcols = np.full((128, P), -1.0, np.float32)
        for p, w in enumerate(wins):
            if w >= 0:
                dcols[:, p] = dinv[gi][w * 128:(w + 1) * 128]
                bcols[:, p] = batch[gi][w * 128:(w + 1) * 128].astype(np.float32)
        m["dinv"] = dcols
        m["dinv_full"] = np.ascontiguousarray(
            dinv[gi].reshape(NW, 128).T).astype(np.float32) / WSCALE
        m["batchrel"] = bcols.astype(bf16)
        cnts = np.bincount(batch[gi], minlength=B).astype(np.float32)
        m["cinv"] = (1.0 / cnts).reshape(B, 1)
        m["wT"] = np.ascontiguousarray(
            (gcn_w[gi][sl].T * WSCALE).astype(fp8).reshape(8, 128, FC)
            .transpose(1, 0, 2))                                 # [128, 8, FC]
        m["bias_rep"] = np.tile(gcn_b[gi][sl].astype(np.float32),
                                (128, 1))                        # [128, FC]
        m["fcT"] = np.ascontiguousarray(
            fc_w[gi][:, sl].T.astype(bf16).reshape(4, 128, OD)
            .transpose(1, 0, 2)).copy()                          # [128, 4, OD]
        m["iota_rep"] = np.tile(np.arange(128, dtype=np.float32)[None, None, :],
                                (128, OHB, 1)).astype(bf16)      # [128, OHB, 128]
        m["iota32"] = np.tile(np.arange(32, dtype=np.float32)[None, None, :],
                              (128, OHB, 1)).astype(bf16)        # [128, OHB, 32]
        lsel = np.zeros((128, 128), np.float32)
        for h in range(NH):
            lsel[32 * h, 32 * h:32 * h + DH] = 1.0
        m["lsel"] = lsel.astype(bf16)
        selB = np.zeros((33, 64), np.float32)
        selB[0, 0:32] = 1.0                          # broadcast mean
        selB[32, 32:64] = 1.0                        # broadcast rstd
        m["selB"] = selB

        # transformer slot data (same as baseline)
        for s in range(GPC):
            g = slot_graphs[s][c]
            T = Ts[s]
            Lg = int(L[g])
            m[f"mas{s}"] = np.stack([
                np.ascontiguousarray(inp[nm][g].T).astype(bf16)
                for nm in mas_names])                             # [4, 80, 128]
            S = np.zeros((4, 128, T), np.float32)
            offk = 0
            for k in range(4):
                lk = int(lens[k, g])
                pp = np.arange(lk)
                S[k, pp, offk + pp] = 1.0
                offk += lk
            m[f"S{s}"] = S.astype(bf16)
            maskT = np.zeros((128, T // 128), np.float32)
            tgrid = (np.arange(T).reshape(T // 128, 128).T)
            maskT[:] = np.where(tgrid < Lg, 0.0, NEG)
            m[f"maskT{s}"] = maskT
            mw = np.zeros((1, T), np.float32)
            mw[0, :min(Lg, T)] = 1.0 / Lg
            m[f"meanw{s}"] = mw.astype(bf16)

        # transformer weights (replicated)
        # Heads packed in PAIRS at 32-partition offsets {0,32} (PE base
        # partition must be 0/32/64).  block b = (comp*NL + li)*2 + p,
        # head h = 2p + hh lives at partitions 32*hh..32*hh+DH.
        winp = np.zeros((TD, 12, 64), np.float32)
        binp = np.zeros((64, 12), np.float32)
        for li in range(NL):
            w = inp["attn_in_w"][li]
            b = inp["attn_in_b"][li]
            for comp in range(3):
                for p in range(2):
                    blk = (comp * NL + li) * 2 + p
                    for hh in range(2):
                        h = 2 * p + hh
                        rows = w[comp * TD + h * DH:comp * TD + (h + 1) * DH]
                        winp[:, blk, 32 * hh:32 * hh + DH] = rows.T
                        bias = b[comp * TD + h * DH:comp * TD + (h + 1) * DH]
                        if comp == 0:
                            bias = bias * ISQ
                        binp[32 * hh:32 * hh + DH, blk] = bias
        m["winT"] = winp.astype(bf16)                                 # [32,12,64]
        m["attn_b"] = binp                                            # [64,12]
        wo = np.zeros((128, NL, TD), np.float32)
        for li in range(NL):
            w = inp["attn_out_w"][li]
            for h in range(NH):
                wo[32 * h:32 * h + DH, li, :] = w[:, h * DH:(h + 1) * DH].T
        m["woutT"] = np.ascontiguousarray(
            wo.reshape(128, NL * TD)).astype(bf16)                    # [128, 64]
        m["woutB"] = np.ascontiguousarray(
            inp["attn_out_b"].T).astype(np.float32)                   # [32, 2]
        m["ln_w"] = np.stack([inp["ln1_w"][0], inp["ln2_w"][0],
                              inp["ln1_w"][1], inp["ln2_w"][1]],
                             axis=1).astype(np.float32)               # [32, 4]
        m["ln_b"] = np.stack([inp["ln1_b"][0], inp["ln2_b"][0],
                              inp["ln1_b"][1], inp["ln2_b"][1]],
                             axis=1).astype(np.float32)
        m["ff1T"] = np.concatenate(
            [np.ascontiguousarray(inp["ff1_w"][li].T) for li in range(NL)],
            axis=1).astype(bf16)                                      # [32, 256]
        m["ff1B"] = np.ascontiguousarray(inp["ff1_b"].T).astype(np.float32)
        m["ff2T"] = np.concatenate(
            [np.ascontiguousarray(inp["ff2_w"][li].T) for li in range(NL)],
            axis=1).astype(bf16)                                      # [128, 64]
        m["ff2B"] = np.ascontiguousarray(inp["ff2_b"].T).astype(np.float32)
        m["redT"] = np.ascontiguousarray(
            np.pad(inp["red_w"].T, ((0, 0), (0, 2)))).astype(bf16)     # [80, 32]
        redb = np.zeros((1, 4 * TD), np.float32)
        for k, (si, fi) in enumerate(inds):
            redb[0, k * TD:k * TD + TD - 2] = inp["red_b"]
            redb[0, k * TD + TD - 2] = si
            redb[0, k * TD + TD - 1] = fi
        m["redb"] = redb.astype(bf16)

    head = dict(
        fc_b=np.stack([np.tile(inp["fc1_b"].astype(np.float32), (B, 1)),
                       np.tile(inp["fc2_b"].astype(np.float32), (B, 1))]),
        fw1=np.ascontiguousarray(inp["final_w"][:, :OD].T).astype(np.float32),
        fw2=np.ascontiguousarray(inp["final_w"][:, OD:2 * OD].T).astype(np.float32),
        fw3=np.ascontiguousarray(inp["final_w"][:, 2 * OD:].T).astype(np.float32),
        fb=np.asarray(inp["final_b"], np.float32).reshape(1, 1),
    )
    return dict(per_core=per_core, head=head, NT=NT, P=P,
                win_of_tile=pos_of_tile, Ts=Ts, slot_graphs=slot_graphs)


# --------------------------------------------------------------------------
# kernel A builder
# --------------------------------------------------------------------------
def _build_a(NT, P, win_of_tile, Ts, repeats=1, stages=3, hw_loop=1,
             gt_bufs=5, interleave=True, tpc=TPC, scratch=16384, nq=4,
             oh_pre=True, sp=False, tw_pre=0.2):
    nc = bacc.Bacc("TRN2", target_bir_lowering=False, debug=False,
                   num_devices=NC, num_swdge_queues=nq,
                   dynamic_dma_scratch_size=scratch)
    dt = {}

    def din(name, shape, dtype):
        dt[name] = nc.dram_tensor(name, shape, dtype, kind="ExternalInput")
        return dt[name]

    din("xtp", [NW, 128, 8, 128], FP8)
    din("wT", [128, 8, FC], FP8)
    din("idx", [128, NT * 128 // 16], I16)
    din("dstrel", [128, NT], BF16)
    din("dinv", [128, P], F32)
    din("dinv_full", [128, NW], F32)
    din("batchrel", [128, P], BF16)
    din("bias_rep", [128, FC], F32)
    din("cinv", [B, 1], F32)
    din("fcT", [128, 4, OD], BF16)
    din("iota_rep", [128, OHB, 128], BF16)
    din("iota32", [128, OHB, 32], BF16)
    din("lsel", [128, 128], BF16)
    din("selB", [33, 64], F32)
    for s in range(GPC):
        din(f"mas{s}", [4, DD, 128], BF16)
        din(f"S{s}", [4, 128, Ts[s]], BF16)
        din(f"maskT{s}", [128, Ts[s] // 128], F32)
        din(f"meanw{s}", [1, Ts[s]], BF16)
    din("winT", [32, 12, 64], BF16)
    din("attn_b", [64, 12], F32)
    din("woutT", [128, 64], BF16)
    din("woutB", [32, 2], F32)
    din("ln_w", [32, 4], F32)
    din("ln_b", [32, 4], F32)
    din("ff1T", [32, 256], BF16)
    din("ff1B", [128, 2], F32)
    din("ff2T", [128, 64], BF16)
    din("ff2B", [32, 2], F32)
    din("redT", [DD, 32], BF16)
    din("redb", [1, 128], BF16)

    po = nc.dram_tensor("po", [B, OD], F32, kind="ExternalOutput")
    masout_d = nc.dram_tensor("masout", [TD, GPC], F32, kind="ExternalOutput")
    gd = nc.dram_tensor("g_scratch", [N, FC], GDT)

    nchunks = (NT + tpc - 1) // tpc
    wot = [int(w) for w in win_of_tile]

    with tile.TileContext(nc) as tc, ExitStack() as ctx:
        const = ctx.enter_context(tc.tile_pool(name="const", bufs=1))
        sb = ctx.enter_context(tc.tile_pool(name="sb", bufs=2))
        sbS = ctx.enter_context(tc.tile_pool(name="sbS", bufs=4))
        sb3 = ctx.enter_context(tc.tile_pool(name="sb3", bufs=4))
        sbg = ctx.enter_context(tc.tile_pool(name="sbg", bufs=gt_bufs))
        sbx = ctx.enter_context(tc.tile_pool(name="sbx", bufs=3))

        def load_const(name):
            src = dt[name][:]
            t = const.tile(list(src.shape), src.dtype, tag=name)
            nc.sync.dma_start(t[:], src)
            return t

        wT_t = load_const("wT")
        dinv_t = load_const("dinv")
        dinvf_t = load_const("dinv_full")
        dstrel_t = load_const("dstrel")
        brel_t = load_const("batchrel")
        bias_t = load_const("bias_rep")
        cinv_t = load_const("cinv")
        fcT_t = load_const("fcT")
        iota_rep_t = load_const("iota_rep")
        iota32_t = load_const("iota32")
        lsel_t = load_const("lsel")
        selB_t = load_const("selB")
        winT_t = load_const("winT")
        attnb_t = load_const("attn_b")
        woutT_t = load_const("woutT")
        woutB_t = load_const("woutB")
        lnw_t = load_const("ln_w")
        lnb_t = load_const("ln_b")
        ff1T_t = load_const("ff1T")
        ff1B_t = load_const("ff1B")
        ff2T_t = load_const("ff2T")
        ff2B_t = load_const("ff2B")
        redT_t = load_const("redT")
        redb_t = load_const("redb")
        maskT_t = [load_const(f"maskT{s}") for s in range(GPC)]
        meanw_t = [load_const(f"meanw{s}") for s in range(GPC)]

        ident = const.tile([128, 128], BF16, tag="ident")
        make_identity(nc, ident[:])
        ones1r = const.tile([1, 128], BF16, tag="ones1r")
        nc.vector.memset(ones1r[:], 1.0)
        ones128 = const.tile([128, 1], BF16, tag="ones128")
        nc.vector.memset(ones128[:], 1.0)
        ones1_32f = const.tile([1, 32], F32, tag="ones1_32f")
        nc.vector.memset(ones1_32f[:], 1.0)
        ones32c = const.tile([32, 1], F32, tag="ones32c")
        nc.vector.memset(ones32c[:], 1.0 / TD)
        ones32b = const.tile([32, 1], BF16, tag="ones32b")
        nc.vector.memset(ones32b[:], 1.0 / TD)
        ones1_32b = const.tile([1, 32], BF16, tag="ones1_32b")
        nc.vector.memset(ones1_32b[:], 1.0)
        eps1 = const.tile([1, 1], F32, tag="eps1")
        nc.vector.memset(eps1[:], EPS)
        ones1_8f = const.tile([1, 8], F32, tag="ones1_8f")
        nc.vector.memset(ones1_8f[:], 1.0)
        masout_sb = const.tile([TD, GPC], F32, tag="masout_sb")
        nc.vector.memset(masout_sb[:], 0.0)

        # idx stream loaded once (big const)
        idx_t = const.tile([128, NT * 128 // 16], I16, tag="idx_t")
        nc.sync.dma_start(idx_t[:], dt["idx"][:])

        # ---- phase 1: g = dinv * (x @ W.T)
        def phase1(psG, twork=(), tw_frac=0.0):
            n_tw = int(len(twork) * tw_frac)
            emitted = 0
            for m in range(NW):
                xt = sbx.tile([128, 8, 128], FP8, tag="xt")
                nc.sync.dma_start(xt[:], dt["xtp"][m])
                hp = psG.tile([128, FC], F32, space="PSUM", tag="scat")
                for kk in range(0, 8, 2):
                    nc.tensor.matmul(hp[:], lhsT=xt[:, kk:kk + 2, :],
                                     rhs=wT_t[:, kk:kk + 2, :],
                                     start=(kk == 0), stop=(kk == 6),
                                     perf_mode=DR)
                gsb = sbx.tile([128, FC], GDT, tag="gsb")
                nc.scalar.activation(gsb[:], hp[:], Act.Identity,
                                     scale=dinvf_t[:, m:m + 1])
                nc.sync.dma_start(gd[m * 128:(m + 1) * 128, :], gsb[:])
                want = (m + 1) * n_tw // NW
                while emitted < want:
                    twork[emitted]()
                    emitted += 1

        # ---- transformer helpers (generator chains, fine-grain rr)
        def lnorm(y_sb, col, T, psT512):
            # original mean->center->var chain, but bf16 operands so the
            # four stat/broadcast matmuls run at 1 cyc/row instead of fp32's 4
            mps = psT512.tile([128, 512], F32, space="PSUM", tag="t512")
            nc.tensor.matmul(mps[:1, :T], lhsT=ones32b[:], rhs=y_sb[:, :T],
                             start=True, stop=True)
            yield
            msb = sb.tile([1, 512], BF16, tag="msb")
            nc.vector.tensor_copy(msb[:1, :T], mps[:1, :T])
            gm = psT512.tile([128, 512], F32, space="PSUM", tag="t512")
            nc.tensor.matmul(gm[:32, :T], lhsT=ones1_32b[:], rhs=msb[:1, :T],
                             start=True, stop=True)
            yield
            yc = sb.tile([32, 512], BF16, tag="yc")
            nc.vector.tensor_tensor(out=yc[:, :T], in0=y_sb[:, :T],
                                    in1=gm[:32, :T], op=Alu.subtract)
            sq = sb.tile([32, 512], BF16, tag="sq")
            nc.vector.tensor_tensor(out=sq[:, :T], in0=yc[:, :T],
                                    in1=yc[:, :T], op=Alu.mult)
            vps = psT512.tile([128, 512], F32, space="PSUM", tag="t512")
            nc.tensor.matmul(vps[:1, :T], lhsT=ones32b[:], rhs=sq[:, :T],
                             start=True, stop=True)
            yield
            sd = sb.tile([1, 512], F32, tag="sd")
            nc.scalar.activation(sd[:1, :T], vps[:1, :T], Act.Sqrt,
                                 bias=eps1[:])
            yield
            rstd = sb.tile([1, 512], BF16, tag="rstd")
            with nc.allow_low_precision(reason="bf16 rstd broadcast"):
                nc.vector.reciprocal(rstd[:1, :T], sd[:1, :T])
            rg = psT512.tile([128, 512], F32, space="PSUM", tag="t512")
            nc.tensor.matmul(rg[:32, :T], lhsT=ones1_32b[:], rhs=rstd[:1, :T],
                             start=True, stop=True)
            yield
            t1 = sb.tile([32, 512], F32, tag="lnt1")
            nc.vector.scalar_tensor_tensor(
                out=t1[:, :T], in0=yc[:, :T], scalar=lnw_t[:, col:col + 1],
                in1=rg[:32, :T], op0=Alu.mult, op1=Alu.mult)
            xo = sbS.tile([32, 512], BF16, tag="lnxo")
            nc.vector.tensor_scalar(out=xo[:, :T], in0=t1[:, :T],
                                    scalar1=lnb_t[:, col:col + 1], scalar2=None,
                                    op0=Alu.add)
            return xo

        def make_twork(psT512, psOps):
            """Transformer emission: one generator per slot, advanced
            round-robin at micro-step granularity so every engine queue
            holds ready work from other chains behind any stalled op.
            Heads packed in pairs at partition offsets {0,32}; softmax
            denominator rides the ops matmul via a ones column at
            partition 32; head outputs collect in SBUF on_all and one
            matmul applies attn_out per (slot, layer)."""

            def chain(s):
                T = Ts[s]
                KT = T // 128
                # ---- pack (accumulator from the ops ring: lives across
                # yields, so it must not occupy the short-lived t512 ring)
                seqps = psOps.tile([33, 512], F32, space="PSUM", tag="ops9")
                for k in range(4):
                    mt = sb.tile([DD, 128], BF16, tag="mt")
                    nc.sync.dma_start(mt[:], dt[f"mas{s}"][k])
                    pp = psT512.tile([128, 512], F32, space="PSUM", tag="t512")
                    nc.tensor.matmul(pp[:, :32], lhsT=mt[:], rhs=redT_t[:],
                                     start=True, stop=False)
                    nc.tensor.matmul(pp[:, :32], lhsT=ones1r[:],
                                     rhs=redb_t[:1, k * TD:(k + 1) * TD],
                                     start=False, stop=True)
                    yield
                    pk = sb.tile([128, 32], BF16, tag="pk")
                    nc.vector.tensor_copy(pk[:], pp[:, :32])
                    Sk = sb.tile([128, 512], BF16, tag="Sk")
                    nc.sync.dma_start(Sk[:, :T], dt[f"S{s}"][k])
                    nc.tensor.matmul(seqps[:TD, :T], lhsT=pk[:],
                                     rhs=Sk[:, :T], start=(k == 0),
                                     stop=(k == 3), skip_group_check=True)
                    yield
                x_sb = sbS.tile([TD, 512], BF16, tag="x_sb")
                nc.scalar.activation(x_sb[:, :T], seqps[:TD, :T],
                                     Act.Identity)
                yield "safe"

                for li in range(NL):
                    # ---- qkv (heads fused per pair)
                    qkv_sb = []
                    for comp in range(3):
                        for p in range(2):
                            blk = (comp * NL + li) * 2 + p
                            cps = psT512.tile([128, 512], F32, space="PSUM",
                                              tag="t512")
                            nc.tensor.matmul(cps[:64, :T],
                                             lhsT=winT_t[:, blk, :],
                                             rhs=x_sb[:, :T], start=True,
                                             stop=True)
                            yield
                            c_sb = sbS.tile([64, 512], BF16,
                                            tag=f"c{comp}{p}_sb")
                            nc.scalar.activation(
                                c_sb[:, :T], cps[:64, :T], Act.Identity,
                                bias=attnb_t[:, blk:blk + 1],
                                scale=(ISQ if comp == 0 else 1.0))
                            qkv_sb.append(c_sb)
                    yield
                    vts = []
                    for p in range(2):
                        # [34*hh : 34*hh+33] per head: cols 0-7 v rows,
                        # 8-31 zeros, col 32 ones -> denom at partition 32.
                        vt_sb = sbS.tile([128, 4, 68], BF16, tag=f"vt{p}_sb")
                        for kt in range(KT):
                            vtp = psT512.tile([128, 512], BF16, space="PSUM",
                                              tag="t512")
                            nc.tensor.transpose(
                                vtp[:, :64],
                                qkv_sb[4 + p][:, kt * 128:(kt + 1) * 128],
                                ident[:64, :64])
                            yield
                            for hh in range(2):
                                nc.vector.tensor_copy(
                                    vt_sb[:, kt, 34 * hh:34 * hh + 32],
                                    vtp[:, 32 * hh:32 * hh + 32])
                        for hh in range(2):
                            nc.vector.memset(
                                vt_sb[:, :KT, 34 * hh + 32:34 * hh + 33], 1.0)
                        vts.append(vt_sb)
                        yield
                    on_all = sbS.tile([128, 512], BF16, tag="on_all")
                    o4 = sb.tile([128, 512], BF16, tag="o4")
                    nc.vector.memset(o4[:], 0.0)
                    linv4 = sb.tile([128, 512], BF16, tag="linv4")
                    yield

                    # ---- attention heads
                    for h in range(NH):
                        p, hh = h // 2, h % 2
                        q_sb, k_sb = qkv_sb[p], qkv_sb[2 + p]
                        vt_sb = vts[p]
                        ops = psOps.tile([33, 512], F32, space="PSUM",
                                         tag="ops9")
                        PT = sb.tile([128, 4, 512], BF16, tag="PT")
                        for kt in range(KT):
                            scp = psT512.tile([128, 512], F32, space="PSUM",
                                              tag="t512")
                            nc.tensor.matmul(
                                scp[:, :T],
                                lhsT=k_sb[32 * hh:32 * hh + 8,
                                          kt * 128:(kt + 1) * 128],
                                rhs=q_sb[32 * hh:32 * hh + 8, :T],
                                start=True, stop=True)
                            yield
                            nc.scalar.activation(PT[:, kt, :T], scp[:, :T],
                                                 Act.Exp,
                                                 bias=maskT_t[s][:, kt:kt + 1])
                            yield
                            nc.tensor.matmul(
                                ops[:, :T],
                                lhsT=vt_sb[:, kt, 34 * hh:34 * hh + 33],
                                rhs=PT[:, kt, :T], start=(kt == 0),
                                stop=(kt == KT - 1), skip_group_check=True)
                            yield
                        nc.scalar.activation(o4[32 * h:32 * h + 8, :T],
                                             ops[:8, :T], Act.Identity)
                        with nc.allow_low_precision(reason="bf16 denom"):
                            nc.vector.reciprocal(
                                linv4[32 * h:32 * h + 1, :T], ops[32:33, :T])
                        yield

                    lg4 = psT512.tile([128, 512], F32, space="PSUM",
                                      tag="t512")
                    nc.tensor.matmul(lg4[:, :T], lhsT=lsel_t[:],
                                     rhs=linv4[:, :T], start=True, stop=True)
                    yield
                    nc.vector.tensor_tensor(out=on_all[:, :T],
                                            in0=o4[:, :T],
                                            in1=lg4[:, :T], op=Alu.mult)
                    yield

                    # ---- post (attn out + residual + ln + ffn + ln)
                    aps = psT512.tile([128, 512], F32, space="PSUM",
                                      tag="t512")
                    nc.tensor.matmul(aps[:TD, :T],
                                     lhsT=woutT_t[:, li * TD:(li + 1) * TD],
                                     rhs=on_all[:, :T], start=True, stop=True)
                    yield
                    y1 = sb.tile([TD, 512], BF16, tag="y1")
                    nc.vector.scalar_tensor_tensor(
                        out=y1[:, :T], in0=aps[:TD, :T],
                        scalar=woutB_t[:, li:li + 1], in1=x_sb[:, :T],
                        op0=Alu.add, op1=Alu.add)
                    yield
                    x_sb = yield from lnorm(y1, 2 * li, T, psT512)
                    f1 = psT512.tile([128, 512], F32, space="PSUM",
                                     tag="t512")
                    nc.tensor.matmul(f1[:, :T],
                                     lhsT=ff1T_t[:, li * DFF:(li + 1) * DFF],
                                     rhs=x_sb[:, :T], start=True, stop=True)
                    yield
                    h1 = sb.tile([DFF, 512], BF16, tag="h1")
                    nc.scalar.activation(h1[:, :T], f1[:, :T], Act.Relu,
                                         bias=ff1B_t[:, li:li + 1])
                    yield
                    f2 = psT512.tile([128, 512], F32, space="PSUM",
                                     tag="t512")
                    nc.tensor.matmul(f2[:32, :T],
                                     lhsT=ff2T_t[:, li * TD:(li + 1) * TD],
                                     rhs=h1[:, :T], start=True, stop=True)
                    yield
                    y2 = sb.tile([TD, 512], BF16, tag="y2")
                    nc.vector.scalar_tensor_tensor(
                        out=y2[:, :T], in0=f2[:32, :T],
                        scalar=ff2B_t[:, li:li + 1],
                        in1=x_sb[:, :T], op0=Alu.add, op1=Alu.add)
                    yield
                    x_sb = yield from lnorm(y2, 2 * li + 1, T, psT512)
                    yield "safe"

                # ---- final masked mean
                mwp = psT512.tile([128, 512], F32, space="PSUM", tag="t512")
                nc.tensor.matmul(mwp[:32, :T], lhsT=ones1_32b[:],
                                 rhs=meanw_t[s][:], start=True, stop=True)
                yield
                mm = sb.tile([TD, 512], F32, tag="mm")
                nc.vector.tensor_tensor(out=mm[:, :T], in0=x_sb[:, :T],
                                        in1=mwp[:32, :T], op=Alu.mult)
                nc.vector.tensor_reduce(out=masout_sb[:, s:s + 1],
                                        in_=mm[:, :T], axis=X, op=Alu.add)

            gens = [chain(s) for s in range(GPC)]
            # pair-staggered driver: two chains run in lockstep (matching
            # the bufs=2 transient rings); at "safe" yields (layer
            # boundaries, only sbS state held) the other pair takes over,
            # so all four slots spread across the whole gather span.
            st = {"pair": 0, "safe": [False] * GPC, "done": [False] * GPC}
            pairs = ((0, 1), (2, 3))

            def round_fn():
                p = pairs[st["pair"]]
                for i in p:
                    if not st["done"][i] and not st["safe"][i]:
                        try:
                            if next(gens[i]) == "safe":
                                st["safe"][i] = True
                        except StopIteration:
                            st["done"][i] = True
                if all(st["done"][i] or st["safe"][i] for i in p):
                    other = 1 - st["pair"]
                    if not all(st["done"][i] for i in pairs[other]):
                        st["pair"] = other
                    for i in pairs[st["pair"]]:
                        st["safe"][i] = False

            def drain_fn():
                while not all(st["done"]):
                    round_fn()

            work = [round_fn] * 300 + [drain_fn]
            return work

        # ---- phase 2: gather + one-hot scatter + pool, interleaved with twork
        def phase2(psG, psPool, twork, gather_only=False):
            pooled = psPool.tile([B, FC], F32, space="PSUM", tag="pooled")
            scat = None
            oh_tiles = {}
            gh_tiles = {}
            emitted = 0
            for q in range(nchunks):
                cs = min(tpc, NT - q * tpc)
                gt = sbg.tile([128, tpc, FC], GDT, tag="gt")
                nc.gpsimd.dma_gather(
                    out_ap=gt[:, :cs, :], in_ap=gd[:],
                    idxs_ap=idx_t[:, q * (tpc * 8):(q * tpc + cs) * 8],
                    num_idxs=cs * 128, num_idxs_reg=cs * 128,
                    elem_size=FC, queue_num=q % nq, single_packet=sp)
                if gather_only:
                    nc.tensor.matmul(pooled[:1, :1], lhsT=gt[:, 0, :1],
                                     rhs=ones128[:], start=(q == 0),
                                     stop=(q == nchunks - 1))
                    if interleave:
                        want = (q + 1) * len(twork) // nchunks
                        while emitted < want and emitted < len(twork):
                            twork[emitted]()
                            emitted += 1
                    continue
                def build_oh(qq):
                    css = min(tpc, NT - qq * tpc)
                    for jb in range(css // OHB):
                        t0 = qq * tpc + jb * OHB
                        oh = sb3.tile([128, OHB, 128], FP8, tag="oh")
                        nc.vector.tensor_tensor(
                            out=oh[:],
                            in0=dstrel_t[:, t0:t0 + OHB].unsqueeze(2)
                            .to_broadcast([128, OHB, 128]),
                            in1=iota_rep_t[:], op=Alu.is_equal)
                        oh_tiles[(qq, jb)] = oh
                if oh_pre:
                    if q == 0:
                        build_oh(0)
                    if q + 1 < nchunks:
                        build_oh(q + 1)
                else:
                    build_oh(q)
                j = 0
                while j < cs:
                    tt = q * tpc + j
                    w = wot[tt]
                    if w < 0:
                        j += 1
                        continue
                    first = (tt == 0) or (wot[tt - 1] != w)
                    pair = (j + 1 < cs and (j % OHB) + 1 < OHB
                            and tt + 1 < NT and wot[tt + 1] == w)
                    if first:
                        scat = psG.tile([128, FC], F32, space="PSUM", tag="scat")
                    if pair:
                        last = (tt + 1 == NT - 1) or (wot[tt + 2] != w)
                        nc.tensor.matmul(
                            scat[:],
                            lhsT=oh_tiles[(q, j // OHB)][:, j % OHB:j % OHB + 2, :],
                            rhs=gt[:, j:j + 2, :], start=first, stop=last,
                            perf_mode=DR)
                        j += 2
                    else:
                        last = (tt == NT - 1) or (wot[tt + 1] != w)
                        nc.tensor.matmul(scat[:],
                                         lhsT=oh_tiles[(q, j // OHB)][:, j % OHB, :],
                                         rhs=gt[:, j, :], start=first, stop=last)
                        j += 1
                    if last:
                        comb = sb3.tile([128, FC], F32, tag="comb")
                        nc.vector.scalar_tensor_tensor(
                            out=comb[:], in0=scat[:],
                            scalar=dinv_t[:, w:w + 1], in1=bias_t[:],
                            op0=Alu.mult, op1=Alu.add)
                        act = sb3.tile([128, FC], BF16, tag="actw")
                        nc.vector.scalar_tensor_tensor(
                            out=act[:], in0=comb[:], scalar=SLOPE, in1=comb[:],
                            op0=Alu.mult, op1=Alu.max)
                        if w % OHB == 0:
                            nb = min(OHB, P - w)
                            ghb = sb3.tile([128, OHB, 32], FP8, tag="ghb")
                            nc.vector.tensor_tensor(
                                out=ghb[:, :nb, :],
                                in0=brel_t[:, w:w + nb].unsqueeze(2)
                                .to_broadcast([128, nb, 32]),
                                in1=iota32_t[:, :nb, :], op=Alu.is_equal)
                            gh_tiles[0] = ghb
                        nc.tensor.matmul(pooled[:], lhsT=gh_tiles[0][:, w % OHB, :],
                                         rhs=act[:], start=(w == 0),
                                         stop=(w == P - 1))
                # interleave transformer emission
                if interleave:
                    want = (q + 1) * len(twork) // nchunks
                    while emitted < want and emitted < len(twork):
                        twork[emitted]()
                        emitted += 1
            while emitted < len(twork):
                twork[emitted]()
                emitted += 1
            return pooled

        def fc_tail(psG, psPool, pooled, psT512):
            pooled_sb = sb.tile([B, FC], BF16, tag="pooled_sb")
            nc.scalar.activation(pooled_sb[:], pooled[:], Act.Identity,
                                 scale=cinv_t[:])
            pooledT = sb.tile([128, 4, 32], BF16, tag="pooledT")
            for k in range(4):
                ptp = psT512.tile([128, 512], BF16, space="PSUM", tag="t512")
                nc.tensor.transpose(ptp[:, :32],
                                    pooled_sb[:, k * 128:(k + 1) * 128],
                                    ident[:B, :B])
                nc.vector.tensor_copy(pooledT[:, k, :], ptp[:, :32])
            fcp = psPool.tile([B, FC], F32, space="PSUM", tag="pooled")
            for k in range(4):
                nc.tensor.matmul(fcp[:, :OD], lhsT=pooledT[:, k, :],
                                 rhs=fcT_t[:, k, :], start=(k == 0),
                                 stop=(k == 3))
            fcsb = sb.tile([B, OD], F32, tag="fcsb")
            nc.vector.tensor_copy(fcsb[:], fcp[:, :OD])
            nc.sync.dma_start(po[:], fcsb[:])

        def body(psG, psPool, psT512, psOps):
            if stages == 5:
                twork = make_twork(psT512, psOps)
                phase1(psG)
                for w in twork:
                    w()
            elif stages == 6:
                twork = make_twork(psT512, psOps)
                phase1(psG)
                phase2(psG, psPool, twork, gather_only=True)
            elif stages == 21:
                phase1(psG)
                phase2(psG, psPool, [], gather_only=True)
            elif stages >= 2:
                twork = make_twork(psT512, psOps) if stages >= 3 else []
                n_pre = int(len(twork) * tw_pre)
                phase1(psG, twork[:n_pre], tw_frac=1.0)
                pooled = phase2(psG, psPool, twork[n_pre:])
                fc_tail(psG, psPool, pooled, psT512)
            else:
                phase1(psG)

        if hw_loop > 1:
            with tc.tile_pool(name="psG", bufs=2, space="PSUM") as psG, \
                 tc.tile_pool(name="psPool", bufs=1, space="PSUM") as psPool, \
                 tc.tile_pool(name="psT512", bufs=2, space="PSUM") as psT512, \
                 tc.tile_pool(name="psOps", bufs=3, space="PSUM") as psOps:
                with tc.For_i(0, hw_loop, 1):
                    body(psG, psPool, psT512, psOps)
        else:
            for _rep in range(repeats):
                with tc.tile_pool(name=f"psG_{_rep}", bufs=2, space="PSUM") as psG, \
                     tc.tile_pool(name=f"psPool_{_rep}", bufs=1, space="PSUM") as psPool, \
                     tc.tile_pool(name=f"psT512_{_rep}", bufs=2, space="PSUM") as psT512, \
                     tc.tile_pool(name=f"psOps_{_rep}", bufs=3, space="PSUM") as psOps:
                    body(psG, psPool, psT512, psOps)
        nc.sync.dma_start(masout_d[:], masout_sb[:])

    nc.compile()
    return nc


# --------------------------------------------------------------------------
# kernel B builder (head)
# --------------------------------------------------------------------------
def _build_b():
    nc = bacc.Bacc("TRN2", target_bir_lowering=False, debug=False,
                   num_devices=NC)
    p1 = nc.dram_tensor("p1", [B, CPG * OD], F32, kind="ExternalInput")
    p2 = nc.dram_tensor("p2", [B, CPG * OD], F32, kind="ExternalInput")
    masT = nc.dram_tensor("masT", [TD, B], F32, kind="ExternalInput")
    fcb = nc.dram_tensor("fcb", [2, B, OD], F32, kind="ExternalInput")
    fw1 = nc.dram_tensor("fw1", [OD, 1], F32, kind="ExternalInput")
    fw2 = nc.dram_tensor("fw2", [OD, 1], F32, kind="ExternalInput")
    fw3 = nc.dram_tensor("fw3", [TD, 1], F32, kind="ExternalInput")
    fb = nc.dram_tensor("fb", [1, 1], F32, kind="ExternalInput")
    y = nc.dram_tensor("y", [B, 1], F32, kind="ExternalOutput")

    with tile.TileContext(nc) as tc, ExitStack() as ctx:
        pool = ctx.enter_context(tc.tile_pool(name="sb", bufs=1))
        psum = ctx.enter_context(tc.tile_pool(name="ps", bufs=2, space="PSUM"))
        ident = pool.tile([B, B], F32, tag="ident")
        make_identity(nc, ident[:])
        ones1_32 = pool.tile([1, B], F32, tag="ones")
        nc.vector.memset(ones1_32[:], 1.0)

        yps = psum.tile([B, 1], F32, space="PSUM", tag="yps")
        for i, (pd, fwd) in enumerate(((p1, fw1), (p2, fw2))):
            pt = pool.tile([B, CPG * OD], F32, tag=f"pt{i}")
            nc.sync.dma_start(pt[:], pd[:])
            acc = pool.tile([B, OD], F32, tag=f"acc{i}")
            nc.vector.tensor_tensor(out=acc[:], in0=pt[:, :OD],
                                    in1=pt[:, OD:2 * OD], op=Alu.add)
            for c in range(2, CPG):
                nc.vector.tensor_tensor(out=acc[:], in0=acc[:],
                                        in1=pt[:, c * OD:(c + 1) * OD],
                                        op=Alu.add)
            fcbt = pool.tile([B, OD], F32, tag=f"fcbt{i}")
            nc.sync.dma_start(fcbt[:], fcb[i])
            nc.vector.tensor_tensor(out=acc[:], in0=acc[:], in1=fcbt[:],
                                    op=Alu.add)
            xl = pool.tile([B, OD], F32, tag=f"xl{i}")
            nc.vector.scalar_tensor_tensor(out=xl[:], in0=acc[:], scalar=SLOPE,
                                           in1=acc[:], op0=Alu.mult, op1=Alu.max)
            xtp = psum.tile([OD, B], F32, space="PSUM", tag=f"xtp{i}")
            nc.tensor.transpose(xtp[:], xl[:], ident[:])
            xt = pool.tile([OD, B], F32, tag=f"xt{i}")
            nc.vector.tensor_copy(xt[:], xtp[:])
            fwt = pool.tile([OD, 1], F32, tag=f"fwt{i}")
            nc.sync.dma_start(fwt[:], fwd[:])
            nc.tensor.matmul(yps[:], lhsT=xt[:], rhs=fwt[:],
                             start=(i == 0), stop=False)
        mt = pool.tile([TD, B], F32, tag="mt")
        nc.sync.dma_start(mt[:], masT[:])
        fw3t = pool.tile([TD, 1], F32, tag="fw3t")
        nc.sync.dma_start(fw3t[:], fw3[:])
        nc.tensor.matmul(yps[:], lhsT=mt[:], rhs=fw3t[:], start=False, stop=False)
        fbt = pool.tile([1, 1], F32, tag="fbt")
        nc.sync.dma_start(fbt[:], fb[:])
        nc.tensor.matmul(yps[:], lhsT=ones1_32[:], rhs=fbt[:],
                         start=False, stop=True)
        ysb = pool.tile([B, 1], F32, tag="ysb")
        nc.vector.tensor_copy(ysb[:], yps[:])
        nc.sync.dma_start(y[:], ysb[:])
    nc.compile()
    return nc


# --------------------------------------------------------------------------
# entry point
# --------------------------------------------------------------------------
def kernel(**inputs) -> np.ndarray:
    prep = _host_prep(inputs)
    key_a = ("A", prep["NT"], prep["P"], tuple(prep["Ts"]),
             tuple(prep["win_of_tile"][:64].tolist()))
    if key_a not in _runner_cache:
        nc_a = _build_a(prep["NT"], prep["P"], prep["win_of_tile"], prep["Ts"])
        _runner_cache[key_a] = _SpmdRunner(nc_a, NC)
    runner_a = _runner_cache[key_a]
    res_a = runner_a.run(prep["per_core"])

    p1 = np.concatenate([res_a[c]["po"] for c in range(CPG)], axis=1)
    p2 = np.concatenate([res_a[c]["po"] for c in range(CPG, NC)], axis=1)
    masT = np.zeros((TD, B), np.float32)
    for c in range(NC):
        for s in range(GPC):
            g = prep["slot_graphs"][s][c]
            masT[:, g] = res_a[c]["masout"][:, s]
    head = prep["head"]
    in_b = dict(p1=p1, p2=p2, masT=masT, fcb=head["fc_b"], fw1=head["fw1"],
                fw2=head["fw2"], fw3=head["fw3"], fb=head["fb"])
    if "B" not in _runner_cache:
        _runner_cache["B"] = _SpmdRunner(_build_b(), NC)
    res_b = _runner_cache["B"].run([in_b] * NC)
    return res_b[0]["y"].astype(np.float32)



# revision 56
# speedup vs baseline: 1.1110x; 1.1110x over previous
"""Trainium2 Bass kernel for nn_GCNN_mutual_attention — v3.

Strategy (8 NeuronCores, SPMD single program, per-core input slices):
  - GCN branches sharded 2 graphs x 2 feature-halves x 2 dst-window-halves:
    core (g, fh, wh) computes table g=[16000,512] fp8 via fp8 DoubleRow
    matmuls (x and W pre-quantized to fp8e4, W scaled by 256 with the
    inverse folded into the per-row dinv scale), then aggregates the edges
    whose dst lies in its window half. dma_gather uses 512B fp8 rows on 4
    SWDGE queues (descriptor-rate bound). Scatter via fp8 one-hot matmuls,
    consecutive same-window tile pairs fused with DoubleRow (2 k-tiles per
    PE instruction). One-hot tiles are built one gather-chunk ahead so the
    scatter's DVE work never waits behind transformer DVE ops.
  - Transformer branch batch-sharded (4 slots/core, rank-strided). Heads
    packed in pairs at partition offsets {0,32} (PE base-partition limit);
    per-head softmax numerator+denominator accumulate in one [33,T] PSUM
    tile via a ones column at partition 32; head outputs collect in SBUF
    (on_all, 32-partition head stride) and a single matmul applies
    attn_out per (slot, layer); the 4 denominators broadcast with one
    selection matmul. Emission is generator-based: two slot-chains run in
    lockstep micro-steps (matching the bufs=2 tile rings) and alternate
    with the other pair at layer boundaries, interleaved into the gather
    chunk stream so PE/ACT/DVE fill the gather window without serial tails.
  - Tiny second launch reduces the 4-core fc partials per graph + head.
"""
import numpy as np
import ml_dtypes
from contextlib import ExitStack

import jax
from jax.sharding import Mesh, PartitionSpec
from jax.experimental.shard_map import shard_map

import concourse.bass as bass
import concourse.tile as tile
import concourse.mybir as mybir
from concourse import bacc
from concourse.bass2jax import _bass_exec_p, install_neuronx_cc_hook, partition_id_tensor
from concourse.masks import make_identity

BF16 = mybir.dt.bfloat16
FP8 = mybir.dt.float8e4
F32 = mybir.dt.float32
I16 = mybir.dt.int16
Alu = mybir.AluOpType
Act = mybir.ActivationFunctionType
X = mybir.AxisListType.X
DR = mybir.MatmulPerfMode.DoubleRow
bf16 = ml_dtypes.bfloat16
fp8 = ml_dtypes.float8_e4m3
WSCALE = 256.0           # fp8 weight pre-scale (keeps 0.02-scale weights normal)

# problem constants
N, F, E, B, OD = 16000, 1024, 256000, 32, 128
DD, TD, NH, DH, DFF, NL = 80, 32, 4, 8, 128, 2
LSUB, MAXLEN = 128, 512
NEG, SLOPE, EPS = -1e9, 0.01, 1e-5
NC = 8
FC = 512                 # feature chunk per core (feat half)
CPG = 4                  # cores per graph
NW = N // 128            # 125 node windows
GPC = B // NC            # transformer graphs per core
TPC = 16                 # gather tiles per chunk (16*128 idx = 2048)
OHB = 8                  # one-hot tiles batched per DVE instr
GDT = FP8                # gather table dtype (BF16 or FP8)
ISQ = float(1.0 / np.sqrt(DH))

_runner_cache = {}


# --------------------------------------------------------------------------
# SPMD runner (reused from baseline)
# --------------------------------------------------------------------------
class _SpmdRunner:
    def __init__(self, nc, n_cores=NC):
        install_neuronx_cc_hook()
        self.n_cores = n_cores
        in_names, out_names, out_avals, zero_outs = [], [], [], []
        pname = nc.partition_id_tensor.name if nc.partition_id_tensor else None
        for alloc in nc.m.functions[0].allocations:
            if not isinstance(alloc, mybir.MemoryLocationSet):
                continue
            name = alloc.memorylocations[0].name
            if alloc.kind == "ExternalInput":
                if name != pname:
                    in_names.append(name)
            elif alloc.kind == "ExternalOutput":
                out_names.append(name)
                out_avals.append(jax.core.ShapedArray(
                    tuple(alloc.tensor_shape), mybir.dt.np(alloc.dtype)))
                zero_outs.append(np.zeros(tuple(alloc.tensor_shape),
                                          mybir.dt.np(alloc.dtype)))
        self.in_names, self.out_names = in_names, out_names
        self.out_avals, self.zero_outs = out_avals, zero_outs
        n_params, n_outs = len(in_names), len(out_avals)
        all_in = list(in_names) + list(out_names)
        if pname is not None:
            all_in.append(pname)

        def _body(*args):
            operands = list(args)
            if pname is not None:
                operands.append(partition_id_tensor())
            return tuple(_bass_exec_p.bind(
                *operands, out_avals=tuple(out_avals), in_names=tuple(all_in),
                out_names=tuple(out_names), lowering_input_output_aliases=(),
                sim_require_finite=True, sim_require_nnan=True, nc=nc))

        devices = jax.devices()[:n_cores]
        self.mesh = Mesh(np.asarray(devices), ("core",))
        in_specs = (PartitionSpec("core"),) * (n_params + n_outs)
        out_specs = (PartitionSpec("core"),) * n_outs
        self.fn = jax.jit(
            shard_map(_body, mesh=self.mesh, in_specs=in_specs,
                      out_specs=out_specs, check_rep=False),
            keep_unused=True)
        self.n_params = n_params

    def prep(self, in_maps):
        per_core = [[np.asarray(m[n]) for n in self.in_names] for m in in_maps]
        concat_in = [np.concatenate([per_core[c][i] for c in range(self.n_cores)],
                                    axis=0) for i in range(self.n_params)]
        concat_zeros = [np.zeros((self.n_cores * z.shape[0], *z.shape[1:]), z.dtype)
                        for z in self.zero_outs]
        return concat_in, concat_zeros

    def run(self, in_maps):
        concat_in, concat_zeros = self.prep(in_maps)
        out_arrs = self.fn(*concat_in, *concat_zeros)
        return [
            {name: np.asarray(out_arrs[i]).reshape(self.n_cores,
                                                   *self.out_avals[i].shape)[c]
             for i, name in enumerate(self.out_names)}
            for c in range(self.n_cores)
        ]


# --------------------------------------------------------------------------
# host-side preprocessing
# --------------------------------------------------------------------------
def _edge_sort(ei):
    """dst-sorted edges incl. self loops, split stats per window."""
    src = np.asarray(ei[0], np.int64)
    dst = np.asarray(ei[1], np.int64)
    deg = np.bincount(dst, minlength=N).astype(np.float64) + 1.0
    dinv = (1.0 / np.sqrt(deg)).astype(np.float32)
    sl = np.arange(N, dtype=np.int64)
    src = np.concatenate([src, sl])
    dst = np.concatenate([dst, sl])
    order = np.argsort(dst, kind="stable")
    s_s, d_s = src[order], dst[order]
    counts = np.bincount(d_s >> 7, minlength=NW)
    return dict(s=s_s, d=d_s, counts=counts, dinv=dinv)


def _half_stream(g, wins, tpp, NT_H):
    """Pack the edges of window list `wins` into the common padded layout.

    tpp[p] = tiles for position p (0 for dummy). Returns idx, dstrel streams.
    """
    ne_pad = NT_H * 128
    src_stream = np.zeros(ne_pad, np.int16)
    dstrel_stream = np.full(ne_pad, -1.0, np.float32)
    off = np.concatenate([[0], np.cumsum(g["counts"])])
    pos = 0
    for p, w in enumerate(wins):
        if w >= 0:
            c = int(g["counts"][w])
            a, b = int(off[w]), int(off[w + 1])
            so = np.argsort(g["s"][a:b], kind="stable")
            src_stream[pos:pos + c] = g["s"][a:b][so]
            dstrel_stream[pos:pos + c] = (g["d"][a:b][so] - (w << 7)).astype(
                np.float32)
        pos += int(tpp[p]) * 128
    idx_np = np.tile(src_stream.reshape(-1, 16).T, (8, 1)).copy()
    dstrel_np = np.ascontiguousarray(
        dstrel_stream.reshape(NT_H, 128).T).astype(bf16)
    return idx_np, dstrel_np


def _host_prep(inp):
    inp = {k: np.asarray(v) for k, v in inp.items()}
    g1 = _edge_sort(inp["pro1_edge_index"])
    g2 = _edge_sort(inp["pro2_edge_index"])
    # split point balancing edge counts (common across graphs)
    cum = np.cumsum(g1["counts"] + g2["counts"])
    WS = int(np.argmin(np.abs(cum - cum[-1] / 2))) + 1
    winsA = list(range(WS))
    winsB = list(range(WS, NW))
    P = max(len(winsA), len(winsB))
    winsA += [-1] * (P - len(winsA))          # dummy positions at end
    winsB += [-1] * (P - len(winsB))
    # common tiles-per-position (dummies get 1 padding tile)
    tpp = np.zeros(P, np.int64)
    for p in range(P):
        cands = []
        for g, wins in ((g1, winsA), (g2, winsB), (g1, winsB), (g2, winsA)):
            w = wins[p]
            cands.append(1 if w < 0 else (int(g["counts"][w]) + 127) // 128)
        tpp[p] = max(cands)
    ntiles = int(tpp.sum())
    NT = ((ntiles + TPC - 1) // TPC) * TPC
    pos_of_tile = np.full(NT, -1, np.int64)
    t = 0
    for p in range(P):
        n = int(tpp[p])
        pos_of_tile[t:t + n] = p
        t += n
    streams = {}
    for gi, g in ((0, g1), (1, g2)):
        for hi, wins in ((0, winsA), (1, winsB)):
            streams[(gi, hi)] = _half_stream(g, wins, tpp, NT)

    def tile_xT(x):
        xT = np.ascontiguousarray(x.T.astype(fp8))             # [F, N]
        tt = xT.reshape(8, 128, NW, 128)                       # [kk, p, m, j]
        return np.ascontiguousarray(tt.transpose(2, 1, 0, 3))  # [m, p, kk, j]

    xtp = [tile_xT(inp["pro1_x"]), tile_xT(inp["pro2_x"])]
    batch = [np.asarray(inp["pro1_batch"], np.int64),
             np.asarray(inp["pro2_batch"], np.int64)]
    gcn_w = [inp["gcn1_w"], inp["gcn2_w"]]
    gcn_b = [inp["gcn1_b"], inp["gcn2_b"]]
    fc_w = [inp["fc1_w"], inp["fc2_w"]]
    dinv = [g1["dinv"], g2["dinv"]]
    wins_of = [winsA, winsB]

    # transformer slot assignment (rank-strided, common padded T per slot)
    lens = np.stack([np.asarray(inp[k + "_lengths"], np.int64) for k in
                     ("mas1_straight", "mas1_flipped", "mas2_straight",
                      "mas2_flipped")])
    L = lens.sum(0)
    rank = np.argsort(-L, kind="stable")
    slot_graphs = [[int(rank[s * NC + c]) for c in range(NC)] for s in range(GPC)]
    Ts = [int(min(MAXLEN, ((int(L[rank[s * NC]]) + 127) // 128) * 128))
          for s in range(GPC)]

    inds = ((1.0, 1.0), (0.0, 1.0), (1.0, 0.0), (0.0, 0.0))
    mas_names = ("mas1_straight", "mas1_flipped", "mas2_straight", "mas2_flipped")

    per_core = [dict() for _ in range(NC)]
    for c in range(NC):
        m = per_core[c]
        gi = c // CPG
        j = c % CPG
        fh, wh = j // 2, j % 2
        sl = slice(fh * FC, (fh + 1) * FC)
        wins = wins_of[wh]
        m["xtp"] = xtp[gi]
        m["idx"], m["dstrel"] = streams[(gi, wh)]
        dcols = np.zeros((128, P), np.float32)
        bcols = np.full((128, P), -1.0, np.float32)
        for p, w in enumerate(wins):
            if w >= 0:
                dcols[:, p] = dinv[gi][w * 128:(w + 1) * 128]
                bcols[:, p] = batch[gi][w * 128:(w + 1) * 128].astype(np.float32)
        m["dinv"] = dcols
        m["dinv_full"] = np.ascontiguousarray(
            dinv[gi].reshape(NW, 128).T).astype(np.float32) / WSCALE
        m["batchrel"] = bcols.astype(bf16)
        cnts = np.bincount(batch[gi], minlength=B).astype(np.float32)
        m["cinv"] = (1.0 / cnts).reshape(B, 1)
        m["wT"] = np.ascontiguousarray(
            (gcn_w[gi][sl].T * WSCALE).astype(fp8).reshape(8, 128, FC)
            .transpose(1, 0, 2))                                 # [128, 8, FC]
        m["bias_rep"] = np.tile(gcn_b[gi][sl].astype(np.float32),
                                (128, 1))                        # [128, FC]
        m["fcT"] = np.ascontiguousarray(
            fc_w[gi][:, sl].T.astype(bf16).reshape(4, 128, OD)
            .transpose(1, 0, 2)).copy()                          # [128, 4, OD]
        m["iota_rep"] = np.tile(np.arange(128, dtype=np.float32)[None, None, :],
                                (128, OHB, 1)).astype(bf16)      # [128, OHB, 128]
        m["iota32"] = np.tile(np.arange(32, dtype=np.float32)[None, None, :],
                              (128, OHB, 1)).astype(bf16)        # [128, OHB, 32]
        lsel = np.zeros((128, 128), np.float32)
        for h in range(NH):
            lsel[32 * h, 32 * h:32 * h + DH] = 1.0
        m["lsel"] = lsel.astype(bf16)
        selB = np.zeros((33, 64), np.float32)
        selB[0, 0:32] = 1.0                          # broadcast mean
        selB[32, 32:64] = 1.0                        # broadcast rstd
        m["selB"] = selB

        # transformer slot data (same as baseline)
        for s in range(GPC):
            g = slot_graphs[s][c]
            T = Ts[s]
            Lg = int(L[g])
            m[f"mas{s}"] = np.stack([
                np.ascontiguousarray(inp[nm][g].T).astype(bf16)
                for nm in mas_names])                             # [4, 80, 128]
            S = np.zeros((4, 128, T), np.float32)
            offk = 0
            for k in range(4):
                lk = int(lens[k, g])
                pp = np.arange(lk)
                S[k, pp, offk + pp] = 1.0
                offk += lk
            m[f"S{s}"] = S.astype(bf16)
            maskT = np.zeros((128, T // 128), np.float32)
            tgrid = (np.arange(T).reshape(T // 128, 128).T)
            maskT[:] = np.where(tgrid < Lg, 0.0, NEG)
            m[f"maskT{s}"] = maskT
            mw = np.zeros((1, T), np.float32)
            mw[0, :min(Lg, T)] = 1.0 / Lg
            m[f"meanw{s}"] = mw.astype(bf16)

        # transformer weights (replicated)
        # Heads packed in PAIRS at 32-partition offsets {0,32} (PE base
        # partition must be 0/32/64).  block b = (comp*NL + li)*2 + p,
        # head h = 2p + hh lives at partitions 32*hh..32*hh+DH.
        winp = np.zeros((TD, 12, 64), np.float32)
        binp = np.zeros((64, 12), np.float32)
        for li in range(NL):
            w = inp["attn_in_w"][li]
            b = inp["attn_in_b"][li]
            for comp in range(3):
                for p in range(2):
                    blk = (comp * NL + li) * 2 + p
                    for hh in range(2):
                        h = 2 * p + hh
                        rows = w[comp * TD + h * DH:comp * TD + (h + 1) * DH]
                        winp[:, blk, 32 * hh:32 * hh + DH] = rows.T
                        bias = b[comp * TD + h * DH:comp * TD + (h + 1) * DH]
                        if comp == 0:
                            bias = bias * ISQ
                        binp[32 * hh:32 * hh + DH, blk] = bias
        m["winT"] = winp.astype(bf16)                                 # [32,12,64]
        m["attn_b"] = binp                                            # [64,12]
        wo = np.zeros((128, NL, TD), np.float32)
        for li in range(NL):
            w = inp["attn_out_w"][li]
            for h in range(NH):
                wo[32 * h:32 * h + DH, li, :] = w[:, h * DH:(h + 1) * DH].T
        m["woutT"] = np.ascontiguousarray(
            wo.reshape(128, NL * TD)).astype(bf16)                    # [128, 64]
        m["woutB"] = np.ascontiguousarray(
            inp["attn_out_b"].T).astype(np.float32)                   # [32, 2]
        m["ln_w"] = np.stack([inp["ln1_w"][0], inp["ln2_w"][0],
                              inp["ln1_w"][1], inp["ln2_w"][1]],
                             axis=1).astype(np.float32)               # [32, 4]
        m["ln_b"] = np.stack([inp["ln1_b"][0], inp["ln2_b"][0],
                              inp["ln1_b"][1], inp["ln2_b"][1]],
                             axis=1).astype(np.float32)
        m["ff1T"] = np.concatenate(
            [np.ascontiguousarray(inp["ff1_w"][li].T) for li in range(NL)],
            axis=1).astype(bf16)                                      # [32, 256]
        m["ff1B"] = np.ascontiguousarray(inp["ff1_b"].T).astype(np.float32)
        m["ff2T"] = np.concatenate(
            [np.ascontiguousarray(inp["ff2_w"][li].T) for li in range(NL)],
            axis=1).astype(bf16)                                      # [128, 64]
        m["ff2B"] = np.ascontiguousarray(inp["ff2_b"].T).astype(np.float32)
        m["redT"] = np.ascontiguousarray(
            np.pad(inp["red_w"].T, ((0, 0), (0, 2)))).astype(bf16)     # [80, 32]
        redb = np.zeros((1, 4 * TD), np.float32)
        for k, (si, fi) in enumerate(inds):
            redb[0, k * TD:k * TD + TD - 2] = inp["red_b"]
            redb[0, k * TD + TD - 2] = si
            redb[0, k * TD + TD - 1] = fi
        m["redb"] = redb.astype(bf16)

    head = dict(
        fc_b=np.stack([np.tile(inp["fc1_b"].astype(np.float32), (B, 1)),
                       np.tile(inp["fc2_b"].astype(np.float32), (B, 1))]),
        fw1=np.ascontiguousarray(inp["final_w"][:, :OD].T).astype(np.float32),
        fw2=np.ascontiguousarray(inp["final_w"][:, OD:2 * OD].T).astype(np.float32),
        fw3=np.ascontiguousarray(inp["final_w"][:, 2 * OD:].T).astype(np.float32),
        fb=np.asarray(inp["final_b"], np.float32).reshape(1, 1),
    )
    return dict(per_core=per_core, head=head, NT=NT, P=P,
                win_of_tile=pos_of_tile, Ts=Ts, slot_graphs=slot_graphs)


# --------------------------------------------------------------------------
# kernel A builder
# --------------------------------------------------------------------------
def _build_a(NT, P, win_of_tile, Ts, repeats=1, stages=3, hw_loop=1,
             gt_bufs=5, interleave=True, tpc=TPC, scratch=16384, nq=4,
             oh_pre=True, sp=False, tw_pre=0.2):
    nc = bacc.Bacc("TRN2", target_bir_lowering=False, debug=False,
                   num_devices=NC, num_swdge_queues=nq,
                   dynamic_dma_scratch_size=scratch)
    dt = {}

    def din(name, shape, dtype):
        dt[name] = nc.dram_tensor(name, shape, dtype, kind="ExternalInput")
        return dt[name]

    din("xtp", [NW, 128, 8, 128], FP8)
    din("wT", [128, 8, FC], FP8)
    din("idx", [128, NT * 128 // 16], I16)
    din("dstrel", [128, NT], BF16)
    din("dinv", [128, P], F32)
    din("dinv_full", [128, NW], F32)
    din("batchrel", [128, P], BF16)
    din("bias_rep", [128, FC], F32)
    din("cinv", [B, 1], F32)
    din("fcT", [128, 4, OD], BF16)
    din("iota_rep", [128, OHB, 128], BF16)
    din("iota32", [128, OHB, 32], BF16)
    din("lsel", [128, 128], BF16)
    din("selB", [33, 64], F32)
    for s in range(GPC):
        din(f"mas{s}", [4, DD, 128], BF16)
        din(f"S{s}", [4, 128, Ts[s]], BF16)
        din(f"maskT{s}", [128, Ts[s] // 128], F32)
        din(f"meanw{s}", [1, Ts[s]], BF16)
    din("winT", [32, 12, 64], BF16)
    din("attn_b", [64, 12], F32)
    din("woutT", [128, 64], BF16)
    din("woutB", [32, 2], F32)
    din("ln_w", [32, 4], F32)
    din("ln_b", [32, 4], F32)
    din("ff1T", [32, 256], BF16)
    din("ff1B", [128, 2], F32)
    din("ff2T", [128, 64], BF16)
    din("ff2B", [32, 2], F32)
    din("redT", [DD, 32], BF16)
    din("redb", [1, 128], BF16)

    po = nc.dram_tensor("po", [B, OD], F32, kind="ExternalOutput")
    masout_d = nc.dram_tensor("masout", [TD, GPC], F32, kind="ExternalOutput")
    gd = nc.dram_tensor("g_scratch", [N, FC], GDT)

    nchunks = (NT + tpc - 1) // tpc
    wot = [int(w) for w in win_of_tile]

    with tile.TileContext(nc) as tc, ExitStack() as ctx:
        const = ctx.enter_context(tc.tile_pool(name="const", bufs=1))
        sb = ctx.enter_context(tc.tile_pool(name="sb", bufs=2))
        sbS = ctx.enter_context(tc.tile_pool(name="sbS", bufs=4))
        sb3 = ctx.enter_context(tc.tile_pool(name="sb3", bufs=4))
        sbg = ctx.enter_context(tc.tile_pool(name="sbg", bufs=gt_bufs))
        sbx = ctx.enter_context(tc.tile_pool(name="sbx", bufs=3))

        def load_const(name):
            src = dt[name][:]
            t = const.tile(list(src.shape), src.dtype, tag=name)
            nc.sync.dma_start(t[:], src)
            return t

        wT_t = load_const("wT")
        dinv_t = load_const("dinv")
        dinvf_t = load_const("dinv_full")
        dstrel_t = load_const("dstrel")
        brel_t = load_const("batchrel")
        bias_t = load_const("bias_rep")
        cinv_t = load_const("cinv")
        fcT_t = load_const("fcT")
        iota_rep_t = load_const("iota_rep")
        iota32_t = load_const("iota32")
        lsel_t = load_const("lsel")
        selB_t = load_const("selB")
        winT_t = load_const("winT")
        attnb_t = load_const("attn_b")
        woutT_t = load_const("woutT")
        woutB_t = load_const("woutB")
        lnw_t = load_const("ln_w")
        lnb_t = load_const("ln_b")
        ff1T_t = load_const("ff1T")
        ff1B_t = load_const("ff1B")
        ff2T_t = load_const("ff2T")
        ff2B_t = load_const("ff2B")
        redT_t = load_const("redT")
        redb_t = load_const("redb")
        maskT_t = [load_const(f"maskT{s}") for s in range(GPC)]
        meanw_t = [load_const(f"meanw{s}") for s in range(GPC)]

        ident = const.tile([128, 128], BF16, tag="ident")
        make_identity(nc, ident[:])
        ones1r = const.tile([1, 128], BF16, tag="ones1r")
        nc.vector.memset(ones1r[:], 1.0)
        ones128 = const.tile([128, 1], BF16, tag="ones128")
        nc.vector.memset(ones128[:], 1.0)
        ones1_32f = const.tile([1, 32], F32, tag="ones1_32f")
        nc.vector.memset(ones1_32f[:], 1.0)
        ones32c = const.tile([32, 1], F32, tag="ones32c")
        nc.vector.memset(ones32c[:], 1.0 / TD)
        ones32b = const.tile([32, 1], BF16, tag="ones32b")
        nc.vector.memset(ones32b[:], 1.0 / TD)
        ones1_32b = const.tile([1, 32], BF16, tag="ones1_32b")
        nc.vector.memset(ones1_32b[:], 1.0)
        eps1 = const.tile([1, 1], F32, tag="eps1")
        nc.vector.memset(eps1[:], EPS)
        ones1_8f = const.tile([1, 8], F32, tag="ones1_8f")
        nc.vector.memset(ones1_8f[:], 1.0)
        masout_sb = const.tile([TD, GPC], F32, tag="masout_sb")
        nc.vector.memset(masout_sb[:], 0.0)

        # idx stream loaded once (big const)
        idx_t = const.tile([128, NT * 128 // 16], I16, tag="idx_t")
        nc.sync.dma_start(idx_t[:], dt["idx"][:])

        # ---- phase 1: g = dinv * (x @ W.T)
        def phase1(psG, twork=(), tw_frac=0.0):
            n_tw = int(len(twork) * tw_frac)
            emitted = 0
            for m in range(NW):
                xt = sbx.tile([128, 8, 128], FP8, tag="xt")
                nc.sync.dma_start(xt[:], dt["xtp"][m])
                hp = psG.tile([128, FC], F32, space="PSUM", tag="scat")
                for kk in range(0, 8, 2):
                    nc.tensor.matmul(hp[:], lhsT=xt[:, kk:kk + 2, :],
                                     rhs=wT_t[:, kk:kk + 2, :],
                                     start=(kk == 0), stop=(kk == 6),
                                     perf_mode=DR)
                gsb = sbx.tile([128, FC], GDT, tag="gsb")
                nc.scalar.activation(gsb[:], hp[:], Act.Identity,
                                     scale=dinvf_t[:, m:m + 1])
                nc.sync.dma_start(gd[m * 128:(m + 1) * 128, :], gsb[:])
                want = (m + 1) * n_tw // NW
                while emitted < want:
                    twork[emitted]()
                    emitted += 1

        # ---- transformer helpers (generator chains, fine-grain rr)
        def lnorm(y_sb, col, T, psT512):
            # original mean->center->var chain, but bf16 operands so the
            # four stat/broadcast matmuls run at 1 cyc/row instead of fp32's 4
            mps = psT512.tile([128, 512], F32, space="PSUM", tag="t512")
            nc.tensor.matmul(mps[:1, :T], lhsT=ones32b[:], rhs=y_sb[:, :T],
                             start=True, stop=True)
            yield
            msb = sb.tile([1, 512], BF16, tag="msb")
            nc.vector.tensor_copy(msb[:1, :T], mps[:1, :T])
            gm = psT512.tile([128, 512], F32, space="PSUM", tag="t512")
            nc.tensor.matmul(gm[:32, :T], lhsT=ones1_32b[:], rhs=msb[:1, :T],
                             start=True, stop=True)
            yield
            yc = sb.tile([32, 512], BF16, tag="yc")
            nc.vector.tensor_tensor(out=yc[:, :T], in0=y_sb[:, :T],
                                    in1=gm[:32, :T], op=Alu.subtract)
            sq = sb.tile([32, 512], BF16, tag="sq")
            nc.vector.tensor_tensor(out=sq[:, :T], in0=yc[:, :T],
                                    in1=yc[:, :T], op=Alu.mult)
            vps = psT512.tile([128, 512], F32, space="PSUM", tag="t512")
            nc.tensor.matmul(vps[:1, :T], lhsT=ones32b[:], rhs=sq[:, :T],
                             start=True, stop=True)
            yield
            sd = sb.tile([1, 512], F32, tag="sd")
            nc.scalar.activation(sd[:1, :T], vps[:1, :T], Act.Sqrt,
                                 bias=eps1[:])
            yield
            rstd = sb.tile([1, 512], BF16, tag="rstd")
            with nc.allow_low_precision(reason="bf16 rstd broadcast"):
                nc.vector.reciprocal(rstd[:1, :T], sd[:1, :T])
            rg = psT512.tile([128, 512], F32, space="PSUM", tag="t512")
            nc.tensor.matmul(rg[:32, :T], lhsT=ones1_32b[:], rhs=rstd[:1, :T],
                             start=True, stop=True)
            yield
            t1 = sb.tile([32, 512], F32, tag="lnt1")
            nc.vector.scalar_tensor_tensor(
                out=t1[:, :T], in0=yc[:, :T], scalar=lnw_t[:, col:col + 1],
                in1=rg[:32, :T], op0=Alu.mult, op1=Alu.mult)
            xo = sbS.tile([32, 512], BF16, tag="lnxo")
            nc.vector.tensor_scalar(out=xo[:, :T], in0=t1[:, :T],
                                    scalar1=lnb_t[:, col:col + 1], scalar2=None,
                                    op0=Alu.add)
            return xo

        def make_twork(psT512, psOps):
            """Transformer emission: one generator per slot, advanced
            round-robin at micro-step granularity so every engine queue
            holds ready work from other chains behind any stalled op.
            Heads packed in pairs at partition offsets {0,32}; softmax
            denominator rides the ops matmul via a ones column at
            partition 32; head outputs collect in SBUF on_all and one
            matmul applies attn_out per (slot, layer)."""

            def chain(s):
                T = Ts[s]
                KT = T // 128
                # ---- pack (accumulator from the ops ring: lives across
                # yields, so it must not occupy the short-lived t512 ring)
                seqps = psOps.tile([33, 512], F32, space="PSUM", tag="ops9")
                for k in range(4):
                    mt = sb.tile([DD, 128], BF16, tag="mt")
                    nc.sync.dma_start(mt[:], dt[f"mas{s}"][k])
                    pp = psT512.tile([128, 512], F32, space="PSUM", tag="t512")
                    nc.tensor.matmul(pp[:, :32], lhsT=mt[:], rhs=redT_t[:],
                                     start=True, stop=False)
                    nc.tensor.matmul(pp[:, :32], lhsT=ones1r[:],
                                     rhs=redb_t[:1, k * TD:(k + 1) * TD],
                                     start=False, stop=True)
                    yield
                    pk = sb.tile([128, 32], BF16, tag="pk")
                    nc.vector.tensor_copy(pk[:], pp[:, :32])
                    Sk = sb.tile([128, 512], BF16, tag="Sk")
                    nc.sync.dma_start(Sk[:, :T], dt[f"S{s}"][k])
                    nc.tensor.matmul(seqps[:TD, :T], lhsT=pk[:],
                                     rhs=Sk[:, :T], start=(k == 0),
                                     stop=(k == 3), skip_group_check=True)
                    yield
                x_sb = sbS.tile([TD, 512], BF16, tag="x_sb")
                nc.scalar.activation(x_sb[:, :T], seqps[:TD, :T],
                                     Act.Identity)
                yield "safe"

                for li in range(NL):
                    # ---- qkv (heads fused per pair)
                    qkv_sb = []
                    for comp in range(3):
                        for p in range(2):
                            blk = (comp * NL + li) * 2 + p
                            cps = psT512.tile([128, 512], F32, space="PSUM",
                                              tag="t512")
                            nc.tensor.matmul(cps[:64, :T],
                                             lhsT=winT_t[:, blk, :],
                                             rhs=x_sb[:, :T], start=True,
                                             stop=True)
                            yield
                            c_sb = sbS.tile([64, 512], BF16,
                                            tag=f"c{comp}{p}_sb")
                            nc.scalar.activation(
                                c_sb[:, :T], cps[:64, :T], Act.Identity,
                                bias=attnb_t[:, blk:blk + 1],
                                scale=(ISQ if comp == 0 else 1.0))
                            qkv_sb.append(c_sb)
                    yield
                    vts = []
                    for p in range(2):
                        # [34*hh : 34*hh+33] per head: cols 0-7 v rows,
                        # 8-31 zeros, col 32 ones -> denom at partition 32.
                        vt_sb = sbS.tile([128, 4, 68], BF16, tag=f"vt{p}_sb")
                        for kt in range(KT):
                            vtp = psT512.tile([128, 512], BF16, space="PSUM",
                                              tag="t512")
                            nc.tensor.transpose(
                                vtp[:, :64],
                                qkv_sb[4 + p][:, kt * 128:(kt + 1) * 128],
                                ident[:64, :64])
                            yield
                            for hh in range(2):
                                nc.vector.tensor_copy(
                                    vt_sb[:, kt, 34 * hh:34 * hh + 32],
                                    vtp[:, 32 * hh:32 * hh + 32])
                        for hh in range(2):
                            nc.vector.memset(
                                vt_sb[:, :KT, 34 * hh + 32:34 * hh + 33], 1.0)
                        vts.append(vt_sb)
                        yield
                    on_all = sbS.tile([128, 512], BF16, tag="on_all")
                    o4 = sb.tile([128, 512], BF16, tag="o4")
                    nc.vector.memset(o4[:], 0.0)
                    linv4 = sb.tile([128, 512], BF16, tag="linv4")
                    yield

                    # ---- attention heads
                    for h in range(NH):
                        p, hh = h // 2, h % 2
                        q_sb, k_sb = qkv_sb[p], qkv_sb[2 + p]
                        vt_sb = vts[p]
                        ops = psOps.tile([33, 512], F32, space="PSUM",
                                         tag="ops9")
                        PT = sb.tile([128, 4, 512], BF16, tag="PT")
                        for kt in range(KT):
                            scp = psT512.tile([128, 512], F32, space="PSUM",
                                              tag="t512")
                            nc.tensor.matmul(
                                scp[:, :T],
                                lhsT=k_sb[32 * hh:32 * hh + 8,
                                          kt * 128:(kt + 1) * 128],
                                rhs=q_sb[32 * hh:32 * hh + 8, :T],
                                start=True, stop=True)
                            yield
                            nc.scalar.activation(PT[:, kt, :T], scp[:, :T],
                                                 Act.Exp,
                                                 bias=maskT_t[s][:, kt:kt + 1])
                            yield
                            nc.tensor.matmul(
                                ops[:, :T],
                                lhsT=vt_sb[:, kt, 34 * hh:34 * hh + 33],
                                rhs=PT[:, kt, :T], start=(kt == 0),
                                stop=(kt == KT - 1), skip_group_check=True)
                            yield
                        nc.scalar.activation(o4[32 * h:32 * h + 8, :T],
                                             ops[:8, :T], Act.Identity)
                        with nc.allow_low_precision(reason="bf16 denom"):
                            nc.vector.reciprocal(
                                linv4[32 * h:32 * h + 1, :T], ops[32:33, :T])
                        yield

                    lg4 = psT512.tile([128, 512], F32, space="PSUM",
                                      tag="t512")
                    nc.tensor.matmul(lg4[:, :T], lhsT=lsel_t[:],
                                     rhs=linv4[:, :T], start=True, stop=True)
                    yield
                    nc.vector.tensor_tensor(out=on_all[:, :T],
                                            in0=o4[:, :T],
                                            in1=lg4[:, :T], op=Alu.mult)
                    yield

                    # ---- post (attn out + residual + ln + ffn + ln)
                    aps = psT512.tile([128, 512], F32, space="PSUM",
                                      tag="t512")
                    nc.tensor.matmul(aps[:TD, :T],
                                     lhsT=woutT_t[:, li * TD:(li + 1) * TD],
                                     rhs=on_all[:, :T], start=True, stop=True)
                    yield
                    y1 = sb.tile([TD, 512], BF16, tag="y1")
                    nc.vector.scalar_tensor_tensor(
                        out=y1[:, :T], in0=aps[:TD, :T],
                        scalar=woutB_t[:, li:li + 1], in1=x_sb[:, :T],
                        op0=Alu.add, op1=Alu.add)
                    yield
                    x_sb = yield from lnorm(y1, 2 * li, T, psT512)
                    f1 = psT512.tile([128, 512], F32, space="PSUM",
                                     tag="t512")
                    nc.tensor.matmul(f1[:, :T],
                                     lhsT=ff1T_t[:, li * DFF:(li + 1) * DFF],
                                     rhs=x_sb[:, :T], start=True, stop=True)
                    yield
                    h1 = sb.tile([DFF, 512], BF16, tag="h1")
                    nc.scalar.activation(h1[:, :T], f1[:, :T], Act.Relu,
                                         bias=ff1B_t[:, li:li + 1])
                    yield
                    f2 = psT512.tile([128, 512], F32, space="PSUM",
                                     tag="t512")
                    nc.tensor.matmul(f2[:32, :T],
                                     lhsT=ff2T_t[:, li * TD:(li + 1) * TD],
                                     rhs=h1[:, :T], start=True, stop=True)
                    yield
                    y2 = sb.tile([TD, 512], BF16, tag="y2")
                    nc.vector.scalar_tensor_tensor(
                        out=y2[:, :T], in0=f2[:32, :T],
                        scalar=ff2B_t[:, li:li + 1],
                        in1=x_sb[:, :T], op0=Alu.add, op1=Alu.add)
                    yield
                    x_sb = yield from lnorm(y2, 2 * li + 1, T, psT512)
                    yield "safe"

                # ---- final masked mean
                mwp = psT512.tile([128, 512], F32, space="PSUM", tag="t512")
                nc.tensor.matmul(mwp[:32, :T], lhsT=ones1_32b[:],
                                 rhs=meanw_t[s][:], start=True, stop=True)
                yield
                mm = sb.tile([TD, 512], F32, tag="mm")
                nc.vector.tensor_tensor(out=mm[:, :T], in0=x_sb[:, :T],
                                        in1=mwp[:32, :T], op=Alu.mult)
                nc.vector.tensor_reduce(out=masout_sb[:, s:s + 1],
                                        in_=mm[:, :T], axis=X, op=Alu.add)

            gens = [chain(s) for s in range(GPC)]
            # pair-staggered driver: two chains run in lockstep (matching
            # the bufs=2 transient rings); at "safe" yields (layer
            # boundaries, only sbS state held) the other pair takes over,
            # so all four slots spread across the whole gather span.
            st = {"pair": 0, "safe": [False] * GPC, "done": [False] * GPC}
            pairs = ((0, 1), (2, 3))

            def round_fn():
                p = pairs[st["pair"]]
                for i in p:
                    if not st["done"][i] and not st["safe"][i]:
                        try:
                            if next(gens[i]) == "safe":
                                st["safe"][i] = True
                        except StopIteration:
                            st["done"][i] = True
                if all(st["done"][i] or st["safe"][i] for i in p):
                    other = 1 - st["pair"]
                    if not all(st["done"][i] for i in pairs[other]):
                        st["pair"] = other
                    for i in pairs[st["pair"]]:
                        st["safe"][i] = False

            def drain_fn():
                while not all(st["done"]):
                    round_fn()

            work = [round_fn] * 300 + [drain_fn]
            return work

        # ---- phase 2: gather + one-hot scatter + pool, interleaved with twork
        def phase2(psG, psPool, twork, gather_only=False):
            pooled = psPool.tile([B, FC], F32, space="PSUM", tag="pooled")
            scat = None
            oh_tiles = {}
            gh_tiles = {}
            emitted = 0
            for q in range(nchunks):
                cs = min(tpc, NT - q * tpc)
                gt = sbg.tile([128, tpc, FC], GDT, tag="gt")
                nc.gpsimd.dma_gather(
                    out_ap=gt[:, :cs, :], in_ap=gd[:],
                    idxs_ap=idx_t[:, q * (tpc * 8):(q * tpc + cs) * 8],
                    num_idxs=cs * 128, num_idxs_reg=cs * 128,
                    elem_size=FC, queue_num=q % nq, single_packet=sp)
                if gather_only:
                    nc.tensor.matmul(pooled[:1, :1], lhsT=gt[:, 0, :1],
                                     rhs=ones128[:], start=(q == 0),
                                     stop=(q == nchunks - 1))
                    if interleave:
                        want = (q + 1) * len(twork) // nchunks
                        while emitted < want and emitted < len(twork):
                            twork[emitted]()
                            emitted += 1
                    continue
                def build_oh(qq):
                    css = min(tpc, NT - qq * tpc)
                    for jb in range(css // OHB):
                        t0 = qq * tpc + jb * OHB
                        oh = sb3.tile([128, OHB, 128], FP8, tag="oh")
                        nc.vector.tensor_tensor(
                            out=oh[:],
                            in0=dstrel_t[:, t0:t0 + OHB].unsqueeze(2)
                            .to_broadcast([128, OHB, 128]),
                            in1=iota_rep_t[:], op=Alu.is_equal)
                        oh_tiles[(qq, jb)] = oh
                if oh_pre:
                    if q == 0:
                        build_oh(0)
                    if q + 1 < nchunks:
                        build_oh(q + 1)
                else:
                    build_oh(q)
                j = 0
                while j < cs:
                    tt = q * tpc + j
                    w = wot[tt]
                    if w < 0:
                        j += 1
                        continue
                    first = (tt == 0) or (wot[tt - 1] != w)
                    pair = (j + 1 < cs and (j % OHB) + 1 < OHB
                            and tt + 1 < NT and wot[tt + 1] == w)
                    if first:
                        scat = psG.tile([128, FC], F32, space="PSUM", tag="scat")
                    if pair:
                        last = (tt + 1 == NT - 1) or (wot[tt + 2] != w)
                        nc.tensor.matmul(
                            scat[:],
                            lhsT=oh_tiles[(q, j // OHB)][:, j % OHB:j % OHB + 2, :],
                            rhs=gt[:, j:j + 2, :], start=first, stop=last,
                            perf_mode=DR)
                        j += 2
                    else:
                        last = (tt == NT - 1) or (wot[tt + 1] != w)
                        nc.tensor.matmul(scat[:],
                                         lhsT=oh_tiles[(q, j // OHB)][:, j % OHB, :],
                                         rhs=gt[:, j, :], start=first, stop=last)
                        j += 1
                    if last:
                        comb = sb3.tile([128, FC], F32, tag="comb")
                        nc.vector.scalar_tensor_tensor(
                            out=comb[:], in0=scat[:],
                            scalar=dinv_t[:, w:w + 1], in1=bias_t[:],
                            op0=Alu.mult, op1=Alu.add)
                        act = sb3.tile([128, FC], BF16, tag="actw")
                        nc.vector.scalar_tensor_tensor(
                            out=act[:], in0=comb[:], scalar=SLOPE, in1=comb[:],
                            op0=Alu.mult, op1=Alu.max)
                        if w % OHB == 0:
                            nb = min(OHB, P - w)
                            ghb = sb3.tile([128, OHB, 32], FP8, tag="ghb")
                            nc.vector.tensor_tensor(
                                out=ghb[:, :nb, :],
                                in0=brel_t[:, w:w + nb].unsqueeze(2)
                                .to_broadcast([128, nb, 32]),
                                in1=iota32_t[:, :nb, :], op=Alu.is_equal)
                            gh_tiles[0] = ghb
                        nc.tensor.matmul(pooled[:], lhsT=gh_tiles[0][:, w % OHB, :],
                                         rhs=act[:], start=(w == 0),
                                         stop=(w == P - 1))
                # interleave transformer emission
                if interleave:
                    want = (q + 1) * len(twork) // nchunks
                    while emitted < want and emitted < len(twork):
                        twork[emitted]()
                        emitted += 1
            while emitted < len(twork):
                twork[emitted]()
                emitted += 1
            return pooled

        def fc_tail(psG, psPool, pooled, psT512):
            pooled_sb = sb.tile([B, FC], BF16, tag="pooled_sb")
            nc.scalar.activation(pooled_sb[:], pooled[:], Act.Identity,
                                 scale=cinv_t[:])
            pooledT = sb.tile([128, 4, 32], BF16, tag="pooledT")
            for k in range(4):
                ptp = psT512.tile([128, 512], BF16, space="PSUM", tag="t512")
                nc.tensor.transpose(ptp[:, :32],
                                    pooled_sb[:, k * 128:(k + 1) * 128],
                                    ident[:B, :B])
                nc.vector.tensor_copy(pooledT[:, k, :], ptp[:, :32])
            fcp = psPool.tile([B, FC], F32, space="PSUM", tag="pooled")
            for k in range(4):
                nc.tensor.matmul(fcp[:, :OD], lhsT=pooledT[:, k, :],
                                 rhs=fcT_t[:, k, :], start=(k == 0),
                                 stop=(k == 3))
            fcsb = sb.tile([B, OD], F32, tag="fcsb")
            nc.vector.tensor_copy(fcsb[:], fcp[:, :OD])
            nc.sync.dma_start(po[:], fcsb[:])

        def body(psG, psPool, psT512, psOps):
            if stages == 5:
                twork = make_twork(psT512, psOps)
                phase1(psG)
                for w in twork:
                    w()
            elif stages == 6:
                twork = make_twork(psT512, psOps)
                phase1(psG)
                phase2(psG, psPool, twork, gather_only=True)
            elif stages == 21:
                phase1(psG)
                phase2(psG, psPool, [], gather_only=True)
            elif stages >= 2:
                twork = make_twork(psT512, psOps) if stages >= 3 else []
                n_pre = int(len(twork) * tw_pre)
                phase1(psG, twork[:n_pre], tw_frac=1.0)
                pooled = phase2(psG, psPool, twork[n_pre:])
                fc_tail(psG, psPool, pooled, psT512)
            else:
                phase1(psG)

        if hw_loop > 1:
            with tc.tile_pool(name="psG", bufs=2, space="PSUM") as psG, \
                 tc.tile_pool(name="psPool", bufs=1, space="PSUM") as psPool, \
                 tc.tile_pool(name="psT512", bufs=2, space="PSUM") as psT512, \
                 tc.tile_pool(name="psOps", bufs=3, space="PSUM") as psOps:
                with tc.For_i(0, hw_loop, 1):
                    body(psG, psPool, psT512, psOps)
        else:
            for _rep in range(repeats):
                with tc.tile_pool(name=f"psG_{_rep}", bufs=2, space="PSUM") as psG, \
                     tc.tile_pool(name=f"psPool_{_rep}", bufs=1, space="PSUM") as psPool, \
                     tc.tile_pool(name=f"psT512_{_rep}", bufs=2, space="PSUM") as psT512, \
                     tc.tile_pool(name=f"psOps_{_rep}", bufs=3, space="PSUM") as psOps:
                    body(psG, psPool, psT512, psOps)
        nc.sync.dma_start(masout_d[:], masout_sb[:])

    nc.compile()
    return nc


# --------------------------------------------------------------------------
# kernel B builder (head)
# --------------------------------------------------------------------------
def _build_b():
    nc = bacc.Bacc("TRN2", target_bir_lowering=False, debug=False,
                   num_devices=NC)
    p1 = nc.dram_tensor("p1", [B, CPG * OD], F32, kind="ExternalInput")
    p2 = nc.dram_tensor("p2", [B, CPG * OD], F32, kind="ExternalInput")
    masT = nc.dram_tensor("masT", [TD, B], F32, kind="ExternalInput")
    fcb = nc.dram_tensor("fcb", [2, B, OD], F32, kind="ExternalInput")
    fw1 = nc.dram_tensor("fw1", [OD, 1], F32, kind="ExternalInput")
    fw2 = nc.dram_tensor("fw2", [OD, 1], F32, kind="ExternalInput")
    fw3 = nc.dram_tensor("fw3", [TD, 1], F32, kind="ExternalInput")
    fb = nc.dram_tensor("fb", [1, 1], F32, kind="ExternalInput")
    y = nc.dram_tensor("y", [B, 1], F32, kind="ExternalOutput")

    with tile.TileContext(nc) as tc, ExitStack() as ctx:
        pool = ctx.enter_context(tc.tile_pool(name="sb", bufs=1))
        psum = ctx.enter_context(tc.tile_pool(name="ps", bufs=2, space="PSUM"))
        ident = pool.tile([B, B], F32, tag="ident")
        make_identity(nc, ident[:])
        ones1_32 = pool.tile([1, B], F32, tag="ones")
        nc.vector.memset(ones1_32[:], 1.0)

        yps = psum.tile([B, 1], F32, space="PSUM", tag="yps")
        for i, (pd, fwd) in enumerate(((p1, fw1), (p2, fw2))):
            pt = pool.tile([B, CPG * OD], F32, tag=f"pt{i}")
            nc.sync.dma_start(pt[:], pd[:])
            acc = pool.tile([B, OD], F32, tag=f"acc{i}")
            nc.vector.tensor_tensor(out=acc[:], in0=pt[:, :OD],
                                    in1=pt[:, OD:2 * OD], op=Alu.add)
            for c in range(2, CPG):
                nc.vector.tensor_tensor(out=acc[:], in0=acc[:],
                                        in1=pt[:, c * OD:(c + 1) * OD],
                                        op=Alu.add)
            fcbt = pool.tile([B, OD], F32, tag=f"fcbt{i}")
            nc.sync.dma_start(fcbt[:], fcb[i])
            nc.vector.tensor_tensor(out=acc[:], in0=acc[:], in1=fcbt[:],
                                    op=Alu.add)
            xl = pool.tile([B, OD], F32, tag=f"xl{i}")
            nc.vector.scalar_tensor_tensor(out=xl[:], in0=acc[:], scalar=SLOPE,
                                           in1=acc[:], op0=Alu.mult, op1=Alu.max)
            xtp = psum.tile([OD, B], F32, space="PSUM", tag=f"xtp{i}")
            nc.tensor.transpose(xtp[:], xl[:], ident[:])
            xt = pool.tile([OD, B], F32, tag=f"xt{i}")
            nc.vector.tensor_copy(xt[:], xtp[:])
            fwt = pool.tile([OD, 1], F32, tag=f"fwt{i}")
            nc.sync.dma_start(fwt[:], fwd[:])
            nc.tensor.matmul(yps[:], lhsT=xt[:], rhs=fwt[:],
                             start=(i == 0), stop=False)
        mt = pool.tile([TD, B], F32, tag="mt")
        nc.sync.dma_start(mt[:], masT[:])
        fw3t = pool.tile([TD, 1], F32, tag="fw3t")
        nc.sync.dma_start(fw3t[:], fw3[:])
        nc.tensor.matmul(yps[:], lhsT=mt[:], rhs=fw3t[:], start=False, stop=False)
        fbt = pool.tile([1, 1], F32, tag="fbt")
        nc.sync.dma_start(fbt[:], fb[:])
        nc.tensor.matmul(yps[:], lhsT=ones1_32[:], rhs=fbt[:],
                         start=False, stop=True)
        ysb = pool.tile([B, 1], F32, tag="ysb")
        nc.vector.tensor_copy(ysb[:], yps[:])
        nc.sync.dma_start(y[:], ysb[:])
    nc.compile()
    return nc


# --------------------------------------------------------------------------
# entry point
# --------------------------------------------------------------------------
def kernel(**inputs) -> np.ndarray:
    prep = _host_prep(inputs)
    key_a = ("A", prep["NT"], prep["P"], tuple(prep["Ts"]),
             tuple(prep["win_of_tile"][:64].tolist()))
    if key_a not in _runner_cache:
        nc_a = _build_a(prep["NT"], prep["P"], prep["win_of_tile"], prep["Ts"])
        _runner_cache[key_a] = _SpmdRunner(nc_a, NC)
    runner_a = _runner_cache[key_a]
    res_a = runner_a.run(prep["per_core"])

    p1 = np.concatenate([res_a[c]["po"] for c in range(CPG)], axis=1)
    p2 = np.concatenate([res_a[c]["po"] for c in range(CPG, NC)], axis=1)
    masT = np.zeros((TD, B), np.float32)
    for c in range(NC):
        for s in range(GPC):
            g = prep["slot_graphs"][s][c]
            masT[:, g] = res_a[c]["masout"][:, s]
    head = prep["head"]
    in_b = dict(p1=p1, p2=p2, masT=masT, fcb=head["fc_b"], fw1=head["fw1"],
                fw2=head["fw2"], fw3=head["fw3"], fb=head["fb"])
    if "B" not in _runner_cache:
        _runner_cache["B"] = _SpmdRunner(_build_b(), NC)
    res_b = _runner_cache["B"].run([in_b] * NC)
    return res_b[0]["y"].astype(np.float32)

